# revision 1
# baseline (speedup 1.0000x reference)
"""Bass/Trainium2 kernel for a 2-layer GAT (GATConv x2 + log_softmax) on 8 NeuronCores.

Strategy (edge/data parallel, dst-sharded, v2):
  - Nodes sharded 8 ways by id (padded shard sh = 128*ceil(N/1024)); core c
    owns dst nodes [c*sh, (c+1)*sh).
  - Phase A is REPLICATED: every core computes h|el for ALL nodes into its
    own local htab (no collective).  Each core's htab/featT are ROTATED by
    c*sh so the first nchunk tiles are its own dst shard -- er (attention
    right-term) for local chunks is kept in SBUF, never in HBM.
  - L1 edge phase (edges partitioned by dst owner, sorted by (block of 2
    chunks, src window, chunk)): per 128-edge tile, dma_gather h[src] rows
    (512B) from htab windows; per-edge er comes from a transposed one-hot
    PE matmul BT[slot,e] @ er_chunk (no er gather); aggregate messages with
    PE matmuls psum[slot,:] += B^T @ [exp(leaky(el+er)) * h[src] | exp(..)].
  - L2 node work fused per chunk: hh|el2|er2 = h2T @ [W2|W2@al2|W2@ar2];
    hh|el2 rows -> l2shard, AllGather -> l2tab (global node order); er2
    stays in SBUF.  L2 edge phase repeats the pipeline on 256B rows, then
    log_softmax into the output shard.
  - Gathers use 4 SWDGE queues round-robin; gather idx are int16 against
    32768-row windows (htab is split into per-window tensors so gathers can
    start before all of phase A finishes).
"""

import os
import sys

import numpy as np

sys.path.insert(0, "/opt/trn_rl_repo")

# ---------------------------------------------------------------- constants
N_NODES = 100000
F_IN = 256
HID = 16
HEADS = 8
N_CLASSES = 16
NEG_SLOPE = 0.2
NC = 8                      # cores
CH = 128                    # dst nodes per chunk
BLK = 2                     # chunks per block (edges padded per (block,win))
GMAX = 8                    # max 128-edge tiles per dma_gather instruction
ROW1 = 128                  # bf16 per L1 table row (h only; el recomputed)
ROW2 = 128                  # bf16 per L2 table row (hh 16 | el2 1 | pad)
WIN = 32768                 # rows addressable by one int16 gather window
FB = 8                      # phase-A node tiles per DMA batch


def _wrap16(v):
    # [n] -> [128, n//16] int16; idx i at [i%16, i//16], replicated over groups
    n = v.shape[0]
    assert n % 16 == 0
    a = v.reshape(n // 16, 16).T.astype(np.int16)      # [16, n//16]
    return np.ascontiguousarray(np.tile(a, (8, 1)))    # [128, n//16]


def _prep_layer(src, dst, n_nodes, sh, n_pad, nchunk, keyfn, win_bounds):
    """Edge layout for one layer.  keyfn maps global src id -> table row;
    win_bounds (ascending, last == n_pad) defines gather windows."""
    wb = np.asarray(win_bounds, np.int64)
    nwin = len(wb) - 1
    assert (wb[1:] - wb[:-1] <= WIN).all()
    nblk = (nchunk + BLK - 1) // BLK

    per_core = []
    cnt = np.zeros((NC, nblk, nwin), dtype=np.int64)
    for c in range(NC):
        m = (dst >= c * sh) & (dst < (c + 1) * sh)
        es = src[m].astype(np.int64)
        ed = (dst[m] - c * sh).astype(np.int64)
        key = keyfn(es, c)
        chunk = ed // CH
        block = chunk // BLK
        win = np.searchsorted(wb, key, side="right") - 1
        order = np.lexsort((key, chunk, win, block))
        es, ed, key, chunk, block, win = (
            x[order] for x in (es, ed, key, chunk, block, win))
        per_core.append((key, ed, chunk, block, win))
        np.add.at(cnt[c], (block, win), 1)

    T = np.ceil(cnt.max(axis=0) / float(CH)).astype(np.int64)  # [nblk, nwin]
    for b in range(nblk):
        if T[b].sum() == 0:
            T[b, 0] = 1

    seg_off = np.zeros((nblk, nwin), dtype=np.int64)
    gath = []                 # (block, window, tile_off, ntiles)
    blocks = []               # (t0, tb) per block
    toff = 0
    for b in range(nblk):
        t0 = toff
        for w in range(nwin):
            seg_off[b, w] = toff
            if T[b, w]:
                gath.append((b, w, toff, int(T[b, w])))
            toff += int(T[b, w])
        blocks.append((t0, toff - t0))
    ntile = toff
    ne_pad = ntile * CH

    gidx = np.zeros((NC, ne_pad), dtype=np.int16)
    slotB = np.full((NC, ne_pad), -1.0, dtype=np.float32)
    flags = np.zeros((ntile, BLK), dtype=bool)
    for c in range(NC):
        key, ed, chunk, block, win = per_core[c]
        # edges are sorted by (block, win, chunk); place each (b,w) group at
        # its segment offset
        bw = block * nwin + win
        grp_start = np.searchsorted(bw, np.arange(nblk * nwin), side="left")
        grp_end = np.searchsorted(bw, np.arange(nblk * nwin), side="right")
        for b in range(nblk):
            for w in range(nwin):
                a, e = int(grp_start[b * nwin + w]), int(grp_end[b * nwin + w])
                if e == a:
                    continue
                pos = seg_off[b, w] * CH
                n = e - a
                gidx[c, pos:pos + n] = (key[a:e] - wb[win[a:e]]).astype(
                    np.int16)
                kk = chunk[a:e] - b * BLK
                slotB[c, pos:pos + n] = (kk * CH + ed[a:e] % CH).astype(
                    np.float32)
                tl = (pos + np.arange(n)) // CH
                flags[tl, kk] = True

    # per-block matmul emission lists (uniform across cores)
    er_ks = []                # [block][tile] -> list of kk with edges
    agg_tiles = []            # [block][kk] -> stream-ordered tiles
    plane_rng = []            # [block][kk] -> (lo, hi) local tile range
    for b in range(nblk):
        t0, tb = blocks[b]
        ek = []
        for t in range(t0, t0 + tb):
            ks = [kk for kk in range(BLK) if flags[t, kk]]
            ek.append(ks if ks else [0])
        er_ks.append(ek)
        at = []
        pr = []
        for kk in range(BLK):
            tl = [t for t in range(t0, t0 + tb) if flags[t, kk]]
            at.append(tl if tl else [t0])
            need = [t - t0 for t in at[kk]]
            need += [i for i, ks in enumerate(ek) if kk in ks]
            pr.append((min(need), max(need) + 1) if need else None)
        agg_tiles.append(at)
        plane_rng.append(pr)

    return dict(
        nwin=nwin, nblk=nblk, ntile=ntile, ne_pad=ne_pad, gath=gath,
        blocks=blocks, er_ks=er_ks, agg_tiles=agg_tiles,
        plane_rng=plane_rng, gidx=gidx, slotB=slotB,
    )


def host_prep(src, dst, n_nodes=N_NODES, nc=NC, ch=CH, win_edge=None):
    src = np.asarray(src, np.int64)
    dst = np.asarray(dst, np.int64)
    nchunk = (n_nodes + CH * NC - 1) // (CH * NC)
    sh = nchunk * CH
    n_pad = NC * sh

    # Both layers' tables live in a half-split layout so each AllGather can
    # be issued in two pieces: half A = dst slots [0, hsz) of every core
    # (table rows [0, rowsA)), half B = the rest.
    hchunk = (nchunk // (2 * BLK)) * BLK
    hsz = hchunk * CH
    rowsA = NC * hsz

    def key2(es, c):
        co = es // sh
        r = es % sh
        inA = r < hsz
        return np.where(inA, co * hsz + r,
                        rowsA + co * (sh - hsz) + (r - hsz))

    wb2 = (list(range(0, rowsA, WIN)) + [rowsA] if rowsA else [0])
    wb2 += [w for w in range(rowsA + WIN, n_pad, WIN)] + [n_pad]
    wb2 = sorted(set(wb2))
    el = _prep_layer(src, dst, n_nodes, sh, n_pad, nchunk, key2, wb2)
    return dict(n_nodes=n_nodes, sh=sh, n_pad=n_pad, nchunk=nchunk,
                hchunk=hchunk, rowsA=rowsA, wb2=wb2, el=el)


# ------------------------------------------------------------- bass program
def build_program(meta, f_in, hid, heads, n_classes):
    from contextlib import ExitStack

    import concourse.tile as tile
    from concourse import bacc, mybir

    dt = mybir.dt
    f32, bf16, i16 = dt.float32, dt.bfloat16, dt.int16
    AF = mybir.ActivationFunctionType
    OP = mybir.AluOpType
    AX = mybir.AxisListType

    n_pad, sh, nchunk = meta["n_pad"], meta["sh"], meta["nchunk"]
    hd = heads * hid
    kt = f_in // 128
    ntile_a = sh // 128
    nwin = meta["el"]["nwin"]

    nq = int(os.environ.get("BASS_QUEUES", "4"))
    bulk = os.environ.get("BASS_BULK", "0") == "1"
    kphase = os.environ.get("BASS_KPHASE", "full")
    krep = int(os.environ.get("BASS_REPEAT", "1"))

    nc_ = bacc.Bacc("TRN2", target_bir_lowering=False, debug=False,
                    num_devices=NC, num_swdge_queues=nq)
    qctr = [0]

    def next_q():
        q = qctr[0] % nq
        qctr[0] += 1
        return q

    def din(name, shape, dtype):
        return nc_.dram_tensor(name, list(shape), dtype,
                               kind="ExternalInput").ap()

    hw1 = hd + heads                # h | er fused matmul width
    featT = din("featT", [f_in, sh], bf16)
    W1 = din("W1", [f_in, hw1], bf16)
    b1rep = din("b1rep", [128, hd], f32)
    alrep = din("alrep", [128, hd], f32)
    W2a = din("W2a", [hd, n_classes + 2], f32)
    b2rep = din("b2rep", [128, n_classes], f32)
    gidx_d = din("gidx", [128, meta["el"]["ne_pad"] // 16], i16)
    slotB_d = din("slotB", [128, meta["el"]["ntile"]], f32)
    slotF_d = din("slotF", [1, meta["el"]["ne_pad"]], dt.uint8)
    out_d = nc_.dram_tensor("out", [sh, n_classes], f32,
                            kind="ExternalOutput").ap()

    wb2 = meta["wb2"]
    hchunk, rowsA = meta["hchunk"], meta["rowsA"]
    rowsB = n_pad - rowsA
    hszA = hchunk * CH

    def mk_pair(name, row, dtype):
        tabA = (nc_.dram_tensor(f"{name}tabA", [rowsA, row], dtype,
                                addr_space="Shared").ap() if rowsA else None)
        tabB = nc_.dram_tensor(f"{name}tabB", [rowsB, row], dtype,
                               addr_space="Shared").ap()
        shA = (nc_.dram_tensor(f"{name}shardA", [hszA, row], dtype).ap()
               if rowsA else None)
        shB = nc_.dram_tensor(f"{name}shardB", [sh - hszA, row],
                              dtype).ap()
        twin = []
        for w in range(len(wb2) - 1):
            lo, hi = wb2[w], wb2[w + 1]
            if lo < rowsA:
                assert hi <= rowsA
                twin.append((tabA, lo, hi))
            else:
                twin.append((tabB, lo - rowsA, hi - rowsA))
        return tabA, tabB, shA, shB, twin

    htabA, htabB, hshardA, hshardB, hwin = mk_pair("h", ROW1, bf16)
    l2tabA, l2tabB, l2shardA, l2shardB, l2win = mk_pair("l2", ROW2, bf16)

    replica = [list(range(NC))]

    with tile.TileContext(nc_) as tc:
        nc = tc.nc
        with ExitStack() as cctx:
            cpool = cctx.enter_context(tc.tile_pool(name="const", bufs=1))
            w1_sb = cpool.tile([128, kt * hw1], bf16, tag="w1")
            for k in range(kt):
                nc.sync.dma_start(w1_sb[:, k * hw1:(k + 1) * hw1],
                                  W1[k * 128:(k + 1) * 128, :])
            al_sb = cpool.tile([128, hd], f32, tag="al")
            nc.sync.dma_start(al_sb[:], alrep[:])
            b1_sb = cpool.tile([128, hd], f32, tag="b1")
            nc.sync.dma_start(b1_sb[:], b1rep[:])
            w2_sb = cpool.tile([hd, n_classes + 2], f32, tag="w2")
            nc.sync.dma_start(w2_sb[:], W2a[:])
            b2_sb = cpool.tile([128, n_classes], f32, tag="b2")
            nc.sync.dma_start(b2_sb[:], b2rep[:])
            iota2_sb = cpool.tile([128, BLK * 128], f32, tag="iota2")
            nc.gpsimd.iota(iota2_sb[:], pattern=[[1, BLK * 128]], base=0,
                           channel_multiplier=0,
                           allow_small_or_imprecise_dtypes=True)
            iota_p = cpool.tile([128, 1], f32, tag="iotap")
            nc.gpsimd.iota(iota_p[:], pattern=[[0, 1]], base=0,
                           channel_multiplier=1,
                           allow_small_or_imprecise_dtypes=True)
            iota_pk = [iota_p]
            for kk in range(1, BLK):
                t = cpool.tile([128, 1], f32, tag=f"iotap{kk}")
                nc.vector.tensor_scalar_add(t[:], iota_p[:],
                                            float(kk * 128))
                iota_pk.append(t)
            ident_sb = cpool.tile([128, 128], f32, tag="ident")
            nc.vector.tensor_scalar(out=ident_sb[:],
                                    in0=iota2_sb[:, 0:128],
                                    scalar1=iota_p[:], scalar2=None,
                                    op0=OP.is_equal)
            er1_sb = cpool.tile([128, nchunk, heads], bf16, tag="er1")
            er2_sb = cpool.tile([128, nchunk, 1], bf16, tag="er2")

            # ---------------- phase A (sharded; AllGather h in halves) -----
            def ag(shard, tab):
                nc.gpsimd.collective_compute(
                    "AllGather", OP.bypass, replica_groups=replica,
                    ins=[shard.opt()], outs=[tab.opt()])

            def phase_a(actx):
                apool = actx.enter_context(tc.tile_pool(name="phA", bufs=2))
                apsum = actx.enter_context(
                    tc.tile_pool(name="phAps", bufs=4, space="PSUM"))
                t_starts = []
                for s0, s1 in ((0, hchunk), (hchunk, ntile_a)):
                    t_starts += [(t0, min(FB, s1 - t0))
                                 for t0 in range(s0, s1, FB)]
                for (bt0, jn) in t_starts:
                    ft = apool.tile([128, kt, FB * 128], bf16, tag="ft")
                    for k in range(kt):
                        nc.sync.dma_start(
                            ft[:, k, 0:jn * 128],
                            featT[k * 128:(k + 1) * 128,
                                  bt0 * 128:bt0 * 128 + jn * 128])
                    rowb = apool.tile([128, FB, ROW1], bf16, tag="rowb")
                    for j in range(jn):
                        t = bt0 + j
                        ps = apsum.tile([128, hw1], f32, tag="hps")
                        for k in range(kt):
                            nc.tensor.matmul(
                                ps[:], lhsT=ft[:, k, j * 128:(j + 1) * 128],
                                rhs=w1_sb[:, k * hw1:(k + 1) * hw1],
                                start=(k == 0), stop=(k == kt - 1))
                        nc.scalar.copy(rowb[:, j, 0:hd], ps[:, 0:hd])
                        nc.scalar.copy(er1_sb[:, t, :],
                                       ps[:, hd:hw1])
                    r0 = bt0 * 128
                    if r0 < hszA:
                        dst = hshardA[r0:r0 + jn * 128, :]
                    else:
                        dst = hshardB[r0 - hszA:r0 - hszA + jn * 128, :]
                    nc.sync.dma_start(
                        dst.rearrange("(s p) r -> p s r", p=128),
                        rowb[:, 0:jn, :])
                    if rowsA and r0 + jn * 128 == hszA:
                        ag(hshardA, htabA)
                ag(hshardB, htabB)

            # ---------------- edge phases ----------------
            def post_chunk_l1(k, ps, ppost, pps2):
                fw, sw = hd, heads
                den = ppost.tile([128, sw], f32, tag="den")
                nc.vector.tensor_scalar_max(den[:], ps[:, fw:fw + sw], 1e-30)
                rec = ppost.tile([128, sw], f32, tag="rec")
                nc.vector.reciprocal(rec[:], den[:])
                h2 = ppost.tile([128, fw], f32, tag="h2")
                nc.vector.tensor_mul(
                    h2[:].rearrange("p (s d) -> p s d", s=sw),
                    ps[:, 0:fw].rearrange("p (s d) -> p s d", s=sw),
                    rec[:].unsqueeze(2).broadcast_to([128, sw, fw // sw]))
                nc.vector.tensor_add(h2[:], h2[:], b1_sb[:])
                mn = ppost.tile([128, fw], f32, tag="mn")
                nc.vector.tensor_scalar_min(mn[:], h2[:], 0.0)
                nc.scalar.activation(mn[:], mn[:], AF.Exp)
                nc.vector.scalar_tensor_tensor(
                    out=h2[:], in0=h2[:], scalar=0.0,
                    in1=mn[:], op0=OP.max, op1=OP.add)
                nc.vector.tensor_scalar_sub(h2[:], h2[:], 1.0)
                # L2 node phase
                pst = pps2.tile([128, 128], f32, tag="pst")
                nc.tensor.transpose(pst[:], h2[:], ident_sb[:])
                h2T = ppost.tile([128, 128], f32, tag="h2T")
                nc.scalar.copy(h2T[:], pst[:])
                ps2 = pps2.tile([128, n_classes + 2], f32, tag="hh")
                nc.tensor.matmul(ps2[:], lhsT=h2T[:], rhs=w2_sb[:],
                                 start=True, stop=True)
                l2r = ppost.tile([128, ROW2], bf16, tag="l2r")
                nc.scalar.copy(l2r[:, 0:n_classes + 1],
                               ps2[:, 0:n_classes + 1])
                nc.vector.memset(l2r[:, n_classes + 1:ROW2], 0.0)
                nc.scalar.copy(er2_sb[:, k, :],
                               ps2[:, n_classes + 1:n_classes + 2])
                if k < hchunk:
                    nc.sync.dma_start(l2shardA[k * CH:(k + 1) * CH, :],
                                      l2r[:])
                else:
                    kb = k - hchunk
                    nc.sync.dma_start(l2shardB[kb * CH:(kb + 1) * CH, :],
                                      l2r[:])

            def post_chunk_l2(k, ps, ppost):
                fw = n_classes
                den = ppost.tile([128, 1], f32, tag="den2")
                nc.vector.tensor_scalar_max(den[:], ps[:, fw:fw + 1], 1e-30)
                rec = ppost.tile([128, 1], f32, tag="rec2")
                nc.vector.reciprocal(rec[:], den[:])
                xx = ppost.tile([128, fw], f32, tag="xx")
                nc.vector.tensor_scalar(out=xx[:], in0=ps[:, 0:fw],
                                        scalar1=rec[:], scalar2=None,
                                        op0=OP.mult)
                nc.vector.tensor_add(xx[:], xx[:], b2_sb[:])
                rmax = ppost.tile([128, 1], f32, tag="rmax")
                nc.vector.tensor_reduce(out=rmax[:], in_=xx[:],
                                        axis=AX.X, op=OP.max)
                nc.vector.tensor_scalar(out=xx[:], in0=xx[:],
                                        scalar1=rmax[:], scalar2=None,
                                        op0=OP.subtract)
                exs = ppost.tile([128, fw], f32, tag="exs")
                ssum = ppost.tile([128, 1], f32, tag="ssum")
                nc.scalar.activation(exs[:], xx[:], AF.Exp,
                                     accum_out=ssum[:])
                lss = ppost.tile([128, 1], f32, tag="lss")
                nc.scalar.activation(lss[:], ssum[:], AF.Ln)
                nc.vector.tensor_scalar(out=xx[:], in0=xx[:],
                                        scalar1=lss[:], scalar2=None,
                                        op0=OP.subtract)
                nc.sync.dma_start(out_d[k * CH:(k + 1) * CH, :], xx[:])

            def edge_phase(layer, mid_blk=None, mid_cb=None):
                lm = meta["el"]
                if layer == 1:
                    rw, fw, sw, gdt = ROW1, hd, heads, bf16
                    er_sb, twin = er1_sb, hwin
                else:
                    rw, fw, sw, gdt = ROW2, n_classes, 1, bf16
                    er_sb, twin = er2_sb, l2win
                nw = fw + sw
                nblk = lm["nblk"]
                gath, blocks = lm["gath"], lm["blocks"]
                er_ks, agg_tiles = lm["er_ks"], lm["agg_tiles"]

                gblocks = {}
                for (b, w, g0, nt) in gath:
                    gblocks.setdefault(b, []).append((w, g0, nt))

                with ExitStack() as ectx:
                    pool = ectx.enter_context(
                        tc.tile_pool(name=f"edge{layer}", bufs=2))
                    pps = ectx.enter_context(
                        tc.tile_pool(name=f"eps{layer}", bufs=2,
                                     space="PSUM"))
                    ppost = ectx.enter_context(
                        tc.tile_pool(name=f"post{layer}", bufs=2))
                    pps2 = ectx.enter_context(
                        tc.tile_pool(name=f"ep2{layer}", bufs=2,
                                     space="PSUM"))
                    ppsE = ectx.enter_context(
                        tc.tile_pool(name=f"epE{layer}", bufs=2,
                                     space="PSUM"))
                    for b in range(nblk):
                        t0, tb = blocks[b]
                        assert tb * sw <= 512, (tb, sw)
                        segs = gblocks[b]
                        gt = pool.tile([128, tb, rw], gdt, tag="gt")
                        if bulk:
                            tabsrc = htabB if layer == 1 else l2tabB
                            nc.sync.dma_start(
                                gt[:],
                                tabsrc[0:128 * tb, :].rearrange(
                                    "(p s) r -> p s r", p=128))
                        for (w, g0, nt) in segs:
                            if bulk:
                                continue
                            ii = pool.tile([128, nt * 8], i16, tag="gi")
                            nc.sync.dma_start(
                                ii[:], gidx_d[:, g0 * 8:(g0 + nt) * 8])
                            t_, lo, hi = twin[w]
                            tab = t_[lo:hi, :]
                            for s0 in range(0, nt, GMAX):
                                sn = min(GMAX, nt - s0)
                                nc.gpsimd.dma_gather(
                                    out_ap=gt[:, g0 - t0 + s0:
                                              g0 - t0 + s0 + sn, :],
                                    in_ap=tab,
                                    idxs_ap=ii[:, s0 * 8:(s0 + sn) * 8],
                                    num_idxs=sn * 128,
                                    num_idxs_reg=sn * 128, elem_size=rw,
                                    queue_num=next_q())
                        # one-hot builds
                        slF = pool.tile([128, tb * 128], dt.uint8, tag="slF")
                        nc.sync.dma_start(
                            slF[:],
                            slotF_d[0:1, t0 * 128:(t0 + tb) * 128]
                            .broadcast_to([128, tb * 128]))
                        BT = pool.tile([128, BLK, tb * 128], bf16, tag="BT")
                        sl = pool.tile([128, tb], f32, tag="sl")
                        nc.sync.dma_start(sl[:], slotB_d[:, t0:t0 + tb])
                        B = pool.tile([128, tb, BLK * 128], bf16, tag="B")
                        for kk in range(BLK):
                            rng = lm["plane_rng"][b][kk]
                            if rng is None:
                                continue
                            lo, hi = rng
                            nc.vector.tensor_scalar(
                                out=BT[:, kk, lo * 128:hi * 128],
                                in0=slF[:, lo * 128:hi * 128],
                                scalar1=iota_pk[kk][:], scalar2=None,
                                op0=OP.is_equal)
                            nc.vector.tensor_tensor(
                                out=B[:, lo:hi, kk * 128:(kk + 1) * 128],
                                in0=iota2_sb[:, kk * 128:(kk + 1) * 128]
                                .unsqueeze(1)
                                .broadcast_to([128, hi - lo, 128]),
                                in1=sl[:, lo:hi].unsqueeze(2)
                                .broadcast_to([128, hi - lo, 128]),
                                op=OP.is_equal)
                        # per-edge er via transposed one-hot matmul
                        psE = ppsE.tile([128, tb * sw], f32, tag="psE")
                        for t in range(tb):
                            ks = er_ks[b][t]
                            for i, kk in enumerate(ks):
                                k = b * BLK + kk
                                nc.tensor.matmul(
                                    psE[:, t * sw:(t + 1) * sw],
                                    lhsT=BT[:, kk, t * 128:(t + 1) * 128],
                                    rhs=er_sb[:, k, :],
                                    start=(i == 0), stop=(i == len(ks) - 1))
                        ex = pool.tile([128, tb, sw], f32, tag="ex")
                        if layer == 1:
                            tmp = pool.tile([128, tb, hd], bf16, tag="tmpel")
                            nc.vector.tensor_mul(
                                tmp[:], gt[:, :, 0:hd],
                                al_sb[:].unsqueeze(1)
                                .broadcast_to([128, tb, hd]))
                            elv = pool.tile([128, tb, sw], f32, tag="elv")
                            nc.vector.tensor_reduce(
                                out=elv[:],
                                in_=tmp[:].rearrange(
                                    "p t (h d) -> p t h d", h=sw),
                                axis=AX.X, op=OP.add)
                            el_ap = elv[:]
                        else:
                            el_ap = gt[:, :, fw:fw + sw]
                        nc.vector.tensor_add(
                            ex[:], el_ap,
                            psE[:].rearrange("p (t s) -> p t s", s=sw))
                        nc.vector.scalar_tensor_tensor(
                            out=ex[:], in0=ex[:], scalar=NEG_SLOPE,
                            in1=ex[:], op0=OP.mult, op1=OP.max)
                        nc.scalar.activation(ex[:], ex[:], AF.Exp)
                        comb = pool.tile([128, tb, nw], bf16, tag="comb")
                        nc.scalar.copy(comb[:, :, fw:fw + sw], ex[:])
                        ex_in = (comb[:, :, fw:fw + sw] if layer == 1
                                 else ex[:])
                        nc.vector.tensor_mul(
                            comb[:, :, 0:fw].rearrange(
                                "p t (s d) -> p t s d", s=sw),
                            gt[:, :, 0:fw].rearrange(
                                "p t (s d) -> p t s d", s=sw),
                            ex_in.unsqueeze(3)
                            .broadcast_to([128, tb, sw, fw // sw]))
                        for kk in range(BLK):
                            k = b * BLK + kk
                            if k >= nchunk:
                                break
                            tl = agg_tiles[b][kk]
                            ps = pps.tile([128, nw], f32, tag="agg")
                            for j, t in enumerate(tl):
                                nc.tensor.matmul(
                                    ps[:], lhsT=B[:, t - t0,
                                                  kk * 128:(kk + 1) * 128],
                                    rhs=comb[:, t - t0, :],
                                    start=(j == 0), stop=(j == len(tl) - 1))
                            if layer == 1:
                                post_chunk_l1(k, ps, ppost, pps2)
                            else:
                                post_chunk_l2(k, ps, ppost)
                        if mid_blk is not None and b == mid_blk:
                            mid_cb()

            for _rep in range(krep):
                with ExitStack() as actx:
                    phase_a(actx)
                if kphase != "A":
                    if rowsA:
                        edge_phase(1, mid_blk=hchunk // BLK - 1,
                                   mid_cb=lambda: ag(l2shardA, l2tabA))
                    else:
                        edge_phase(1)
                    ag(l2shardB, l2tabB)
                    if kphase != "L1":
                        edge_phase(2)

    nc_.compile()
    return nc_


# ------------------------------------------------------------------ driver
def make_in_maps(meta, feat, W1, al1, ar1, b1, W2, al2, ar2, b2):
    import ml_dtypes
    bf16 = ml_dtypes.bfloat16
    sh, n_pad = meta["sh"], meta["n_pad"]
    n = meta["n_nodes"]
    feat_pad = np.zeros((n_pad, feat.shape[1]), np.float32)
    feat_pad[:n] = feat
    # fused [W1 | W1@al | W1@ar]: el/er are linear in feat
    heads, hid = al1.shape
    w1r = W1.reshape(-1, heads, hid)
    w1ar = np.einsum("fhd,hd->fh", w1r, ar1)
    W1a = np.concatenate([W1, w1ar], axis=1).astype(np.float32)
    alrep = np.tile(
        np.einsum("hd->hd", al1).reshape(1, -1), (128, 1)
    ).astype(np.float32)
    b1rep = np.tile(b1.reshape(1, -1), (128, 1)).astype(np.float32)
    W2a = np.concatenate(
        [W2, W2 @ al2.reshape(-1, 1), W2 @ ar2.reshape(-1, 1)],
        axis=1).astype(np.float32)
    b2rep = np.tile(b2.reshape(1, -1), (128, 1)).astype(np.float32)
    el = meta["el"]
    in_maps = []
    for c in range(NC):
        in_maps.append({
            "featT": np.ascontiguousarray(
                feat_pad[c * sh:(c + 1) * sh].T).astype(bf16),
            "W1": W1a.astype(bf16),
            "b1rep": b1rep, "alrep": alrep,
            "W2a": W2a, "b2rep": b2rep,
            "gidx": _wrap16(el["gidx"][c]),
            "slotB": np.ascontiguousarray(
                el["slotB"][c].reshape(-1, 128).T).astype(np.float32),
            "slotF": np.maximum(el["slotB"][c], 0.0).reshape(1, -1)
            .astype(np.uint8),
        })
    return in_maps


class Runner:
    """Builds the SPMD program once; exposes a repeatable timed executor."""

    def __init__(self, meta, f_in):
        self.meta = meta
        self.nc = build_program(meta, f_in, HID, HEADS, N_CLASSES)
        self._fn = None

    def _lower(self):
        import jax
        import numpy as _np
        from jax.sharding import Mesh, PartitionSpec
        from jax.experimental.shard_map import shard_map
        from concourse import mybir
        from concourse.bass2jax import _bass_exec_p, install_neuronx_cc_hook

        install_neuronx_cc_hook()
        nc = self.nc
        in_names, out_names, out_avals, zero_outs = [], [], [], []
        partition_name = (nc.partition_id_tensor.name
                          if nc.partition_id_tensor else None)
        for alloc in nc.m.functions[0].allocations:
            if not isinstance(alloc, mybir.MemoryLocationSet):
                continue
            name = alloc.memorylocations[0].name
            if alloc.kind == "ExternalInput":
                if name != partition_name:
                    in_names.append(name)
            elif alloc.kind == "ExternalOutput":
                shape = tuple(alloc.tensor_shape)
                dtype = mybir.dt.np(alloc.dtype)
                out_names.append(name)
                out_avals.append(jax.core.ShapedArray(shape, dtype))
                zero_outs.append(_np.zeros(shape, dtype))
        n_params = len(in_names)
        n_outs = len(out_avals)
        all_in_names = list(in_names) + list(out_names)
        if partition_name is not None:
            all_in_names.append(partition_name)

        def _body(*args):
            ins = list(args[:n_params])
            zouts = list(args[n_params:])
            operands = ins + zouts
            if partition_name is not None:
                from concourse.bass2jax import partition_id_tensor
                operands.append(partition_id_tensor())
            outs = _bass_exec_p.bind(
                *operands, out_avals=tuple(out_avals),
                in_names=tuple(all_in_names), out_names=tuple(out_names),
                lowering_input_output_aliases=(),
                sim_require_finite=False, sim_require_nnan=False, nc=nc)
            return tuple(outs)

        devices = jax.devices()[:NC]
        mesh = Mesh(_np.asarray(devices), ("core",))
        in_specs = (PartitionSpec("core"),) * (n_params + n_outs)
        out_specs = (PartitionSpec("core"),) * n_outs
        self._fn = jax.jit(
            shard_map(_body, mesh=mesh, in_specs=in_specs,
                      out_specs=out_specs, check_rep=False),
            keep_unused=True)
        self._in_names = in_names
        self._out_names = out_names
        self._out_avals = out_avals
        self._zero_outs = zero_outs
        self._mesh = mesh
        self._in_specs = in_specs

    def prepare(self, in_maps):
        import jax
        import numpy as _np
        from jax.sharding import NamedSharding, PartitionSpec
        if self._fn is None:
            self._lower()
        concat_in = [
            _np.concatenate([in_maps[c][name] for c in range(NC)], axis=0)
            for name in self._in_names]
        concat_zeros = [
            _np.zeros((NC * z.shape[0], *z.shape[1:]), z.dtype)
            for z in self._zero_outs]
        shd = NamedSharding(self._mesh, PartitionSpec("core"))
        self._args = [jax.device_put(a, shd) for a in concat_in + concat_zeros]
        jax.block_until_ready(self._args)

    def run(self):
        import jax
        out = self._fn(*self._args)
        out = jax.block_until_ready(out)
        import numpy as _np
        res = _np.asarray(out[self._out_names.index("out")])
        shp = self._out_avals[self._out_names.index("out")].shape
        res = res.reshape(NC, *shp)
        n, sh = self.meta["n_nodes"], self.meta["sh"]
        parts = [res[c][:min(sh, max(0, n - c * sh))] for c in range(NC)]
        return _np.concatenate(parts, axis=0)


_RUNNER = None


def get_runner(feat, src, dst):
    global _RUNNER
    n, f_in = feat.shape
    meta = host_prep(np.asarray(src, np.int32), np.asarray(dst, np.int32),
                     n_nodes=n)
    _RUNNER = Runner(meta, f_in)
    return _RUNNER


def kernel(feat, src, dst, W1, al1, ar1, b1, W2, al2, ar2, b2):
    feat = np.asarray(feat, dtype=np.float32)
    src = np.asarray(src, dtype=np.int32)
    dst = np.asarray(dst, dtype=np.int32)
    args = [np.asarray(x, np.float32)
            for x in (W1, al1, ar1, b1, W2, al2, ar2, b2)]
    r = _RUNNER if _RUNNER is not None else get_runner(feat, src, dst)
    in_maps = make_in_maps(r.meta, feat, *args)
    r.prepare(in_maps)
    return r.run()


kernel.last_exec_time_ns = None



# revision 15
# speedup vs baseline: 1.9287x; 1.9287x over previous
"""Bass/Trainium2 kernel for a 2-layer GAT (GATConv x2 + log_softmax) on 8 NeuronCores.

Strategy (edge/data parallel, dst-sharded, v2):
  - Nodes sharded 8 ways by id (padded shard sh = 128*ceil(N/1024)); core c
    owns dst nodes [c*sh, (c+1)*sh).
  - Phase A is REPLICATED: every core computes h|el for ALL nodes into its
    own local htab (no collective).  Each core's htab/featT are ROTATED by
    c*sh so the first nchunk tiles are its own dst shard -- er (attention
    right-term) for local chunks is kept in SBUF, never in HBM.
  - L1 edge phase (edges partitioned by dst owner, sorted by (block of 2
    chunks, src window, chunk)): per 128-edge tile, dma_gather h[src] rows
    (512B) from htab windows; per-edge er comes from a transposed one-hot
    PE matmul BT[slot,e] @ er_chunk (no er gather); aggregate messages with
    PE matmuls psum[slot,:] += B^T @ [exp(leaky(el+er)) * h[src] | exp(..)].
  - L2 node work fused per chunk: hh|el2|er2 = h2T @ [W2|W2@al2|W2@ar2];
    hh|el2 rows -> l2shard, AllGather -> l2tab (global node order); er2
    stays in SBUF.  L2 edge phase repeats the pipeline on 256B rows, then
    log_softmax into the output shard.
  - Gathers use 4 SWDGE queues round-robin; gather idx are int16 against
    32768-row windows (htab is split into per-window tensors so gathers can
    start before all of phase A finishes).
"""

import os
import sys

import numpy as np

sys.path.insert(0, "/opt/trn_rl_repo")

# ---------------------------------------------------------------- constants
N_NODES = 100000
F_IN = 256
HID = 16
HEADS = 8
N_CLASSES = 16
NEG_SLOPE = 0.2
NC = 8                      # cores
CH = 128                    # dst nodes per chunk
BLK = 2                     # chunks per block (edges padded per (block,win))
GMAX = 8                    # max 128-edge tiles per dma_gather instruction
ROW1 = 128                  # bf16 per L1 table row (h only; el recomputed)
ROW2 = 128                  # bf16 per L2 table row (hh 16 | el2 1 | pad)
WIN = 32768                 # rows addressable by one int16 gather window
FB = 8                      # phase-A node tiles per DMA batch


def _wrap16(v):
    # [n] -> [128, n//16] int16; idx i at [i%16, i//16], replicated over groups
    n = v.shape[0]
    assert n % 16 == 0
    a = v.reshape(n // 16, 16).T.astype(np.int16)      # [16, n//16]
    return np.ascontiguousarray(np.tile(a, (8, 1)))    # [128, n//16]


def _prep_layer(src, dst, n_nodes, sh, n_pad, nchunk, keyfn, win_bounds):
    """Edge layout for one layer.  keyfn maps global src id -> table row;
    win_bounds (ascending, last == n_pad) defines gather windows."""
    wb = np.asarray(win_bounds, np.int64)
    nwin = len(wb) - 1
    assert (wb[1:] - wb[:-1] <= WIN).all()
    nblk = (nchunk + BLK - 1) // BLK

    per_core = []
    cnt = np.zeros((NC, nblk, nwin), dtype=np.int64)
    for c in range(NC):
        m = (dst >= c * sh) & (dst < (c + 1) * sh)
        es = src[m].astype(np.int64)
        ed = (dst[m] - c * sh).astype(np.int64)
        key = keyfn(es, c)
        chunk = ed // CH
        block = chunk // BLK
        win = np.searchsorted(wb, key, side="right") - 1
        order = np.lexsort((key, chunk, win, block))
        es, ed, key, chunk, block, win = (
            x[order] for x in (es, ed, key, chunk, block, win))
        per_core.append((key, ed, chunk, block, win))
        np.add.at(cnt[c], (block, win), 1)

    T = np.ceil(cnt.max(axis=0) / float(CH)).astype(np.int64)  # [nblk, nwin]
    for b in range(nblk):
        if T[b].sum() == 0:
            T[b, 0] = 1

    seg_off = np.zeros((nblk, nwin), dtype=np.int64)
    gath = []                 # (block, window, tile_off, ntiles)
    blocks = []               # (t0, tb) per block
    toff = 0
    for b in range(nblk):
        t0 = toff
        for w in range(nwin):
            seg_off[b, w] = toff
            if T[b, w]:
                gath.append((b, w, toff, int(T[b, w])))
            toff += int(T[b, w])
        blocks.append((t0, toff - t0))
    ntile = toff
    ne_pad = ntile * CH

    gidx = np.zeros((NC, ne_pad), dtype=np.int16)
    slotB = np.full((NC, ne_pad), -1.0, dtype=np.float32)
    flags = np.zeros((ntile, BLK), dtype=bool)
    for c in range(NC):
        key, ed, chunk, block, win = per_core[c]
        # edges are sorted by (block, win, chunk); place each (b,w) group at
        # its segment offset
        bw = block * nwin + win
        grp_start = np.searchsorted(bw, np.arange(nblk * nwin), side="left")
        grp_end = np.searchsorted(bw, np.arange(nblk * nwin), side="right")
        for b in range(nblk):
            for w in range(nwin):
                a, e = int(grp_start[b * nwin + w]), int(grp_end[b * nwin + w])
                if e == a:
                    continue
                pos = seg_off[b, w] * CH
                n = e - a
                gidx[c, pos:pos + n] = (key[a:e] - wb[win[a:e]]).astype(
                    np.int16)
                kk = chunk[a:e] - b * BLK
                slotB[c, pos:pos + n] = (kk * CH + ed[a:e] % CH).astype(
                    np.float32)
                tl = (pos + np.arange(n)) // CH
                flags[tl, kk] = True

    # per-block matmul emission lists (uniform across cores)
    er_ks = []                # [block][tile] -> list of kk with edges
    agg_tiles = []            # [block][kk] -> stream-ordered tiles
    plane_rng = []            # [block][kk] -> (lo, hi) local tile range
    bplanes = []              # [block] -> [(kk, t_local)] B one-hot planes
    for b in range(nblk):
        t0, tb = blocks[b]
        ek = []
        for t in range(t0, t0 + tb):
            ks = [kk for kk in range(BLK) if flags[t, kk]]
            ek.append(ks if ks else [0])
        er_ks.append(ek)
        at = []
        pr = []
        bp = []
        for kk in range(BLK):
            tl = [t for t in range(t0, t0 + tb) if flags[t, kk]]
            at.append(tl if tl else [t0])
            need = [t - t0 for t in at[kk]]
            need += [i for i, ks in enumerate(ek) if kk in ks]
            pr.append((min(need), max(need) + 1) if need else None)
            bp += [(kk, t - t0) for t in at[kk]]
        agg_tiles.append(at)
        plane_rng.append(pr)
        bplanes.append(sorted(bp, key=lambda p: (p[1], p[0])))

    return dict(
        nwin=nwin, nblk=nblk, ntile=ntile, ne_pad=ne_pad, gath=gath,
        blocks=blocks, er_ks=er_ks, agg_tiles=agg_tiles,
        plane_rng=plane_rng, bplanes=bplanes, gidx=gidx, slotB=slotB,
    )


def host_prep(src, dst, n_nodes=N_NODES, nc=NC, ch=CH, win_edge=None):
    src = np.asarray(src, np.int64)
    dst = np.asarray(dst, np.int64)
    nchunk = (n_nodes + CH * NC - 1) // (CH * NC)
    sh = nchunk * CH
    n_pad = NC * sh

    # Both layers' tables live in a half-split layout so each AllGather can
    # be issued in two pieces: half A = dst slots [0, hsz) of every core
    # (table rows [0, rowsA)), half B = the rest.
    hchunk = (nchunk // (2 * BLK)) * BLK
    hsz = hchunk * CH
    rowsA = NC * hsz

    def key2(es, c):
        co = es // sh
        r = es % sh
        inA = r < hsz
        return np.where(inA, co * hsz + r,
                        rowsA + co * (sh - hsz) + (r - hsz))

    wb2 = (list(range(0, rowsA, WIN)) + [rowsA] if rowsA else [0])
    wb2 += [w for w in range(rowsA + WIN, n_pad, WIN)] + [n_pad]
    wb2 = sorted(set(wb2))
    el = _prep_layer(src, dst, n_nodes, sh, n_pad, nchunk, key2, wb2)
    return dict(n_nodes=n_nodes, sh=sh, n_pad=n_pad, nchunk=nchunk,
                hchunk=hchunk, rowsA=rowsA, wb2=wb2, el=el)


# ------------------------------------------------------------- bass program
def build_program(meta, f_in, hid, heads, n_classes):
    from contextlib import ExitStack

    import concourse.tile as tile
    from concourse import bacc, mybir

    dt = mybir.dt
    f32, bf16, i16 = dt.float32, dt.bfloat16, dt.int16
    AF = mybir.ActivationFunctionType
    OP = mybir.AluOpType
    AX = mybir.AxisListType

    n_pad, sh, nchunk = meta["n_pad"], meta["sh"], meta["nchunk"]
    hd = heads * hid
    kt = f_in // 128
    ntile_a = sh // 128
    nwin = meta["el"]["nwin"]

    nq = int(os.environ.get("BASS_QUEUES", "4"))
    bulk = os.environ.get("BASS_BULK", "0") == "1"
    kphase = os.environ.get("BASS_KPHASE", "full")
    krep = int(os.environ.get("BASS_REPEAT", "1"))

    nc_ = bacc.Bacc("TRN2", target_bir_lowering=False, debug=False,
                    num_devices=NC, num_swdge_queues=nq)
    qctr = [0]

    def next_q():
        q = qctr[0] % nq
        qctr[0] += 1
        return q

    def din(name, shape, dtype):
        return nc_.dram_tensor(name, list(shape), dtype,
                               kind="ExternalInput").ap()

    hw1 = hd + heads                # h | er fused matmul width
    featT = din("featT", [f_in, sh], bf16)
    W1 = din("W1", [f_in, hw1], bf16)
    b1rep = din("b1rep", [128, hd], f32)
    alrep = din("alrep", [128, hd], bf16)
    W2a = din("W2a", [hd, n_classes + 2], f32)
    b2rep = din("b2rep", [128, n_classes], f32)
    gidx_d = din("gidx", [128, meta["el"]["ne_pad"] // 16], i16)
    slotB_d = din("slotB", [128, meta["el"]["ntile"]], f32)
    slotF_d = din("slotF", [1, meta["el"]["ne_pad"]], bf16)
    out_d = nc_.dram_tensor("out", [sh, n_classes], f32,
                            kind="ExternalOutput").ap()

    wb2 = meta["wb2"]
    hchunk, rowsA = meta["hchunk"], meta["rowsA"]
    rowsB = n_pad - rowsA
    hszA = hchunk * CH

    def mk_pair(name, row, dtype):
        tabA = (nc_.dram_tensor(f"{name}tabA", [rowsA, row], dtype,
                                addr_space="Shared").ap() if rowsA else None)
        tabB = nc_.dram_tensor(f"{name}tabB", [rowsB, row], dtype,
                               addr_space="Shared").ap()
        shA = (nc_.dram_tensor(f"{name}shardA", [hszA, row], dtype).ap()
               if rowsA else None)
        shB = nc_.dram_tensor(f"{name}shardB", [sh - hszA, row],
                              dtype).ap()
        twin = []
        for w in range(len(wb2) - 1):
            lo, hi = wb2[w], wb2[w + 1]
            if lo < rowsA:
                assert hi <= rowsA
                twin.append((tabA, lo, hi))
            else:
                twin.append((tabB, lo - rowsA, hi - rowsA))
        return tabA, tabB, shA, shB, twin

    htabA, htabB, hshardA, hshardB, hwin = mk_pair("h", ROW1, bf16)
    l2tabA, l2tabB, l2shardA, l2shardB, l2win = mk_pair("l2", ROW2, bf16)

    replica = [list(range(NC))]

    with tile.TileContext(nc_) as tc:
        nc = tc.nc
        with ExitStack() as cctx:
            cpool = cctx.enter_context(tc.tile_pool(name="const", bufs=1))
            w1_sb = cpool.tile([128, kt * hw1], bf16, tag="w1")
            for k in range(kt):
                nc.sync.dma_start(w1_sb[:, k * hw1:(k + 1) * hw1],
                                  W1[k * 128:(k + 1) * 128, :])
            al_sb = cpool.tile([128, hd], bf16, tag="al")
            nc.sync.dma_start(al_sb[:], alrep[:])
            b1_sb = cpool.tile([128, hd], f32, tag="b1")
            nc.sync.dma_start(b1_sb[:], b1rep[:])
            w2_sb = cpool.tile([hd, n_classes + 2], f32, tag="w2")
            nc.sync.dma_start(w2_sb[:], W2a[:])
            b2_sb = cpool.tile([128, n_classes], f32, tag="b2")
            nc.sync.dma_start(b2_sb[:], b2rep[:])
            iota2_sb = cpool.tile([128, 128], f32, tag="iota2")
            nc.gpsimd.iota(iota2_sb[:], pattern=[[1, 128]], base=0,
                           channel_multiplier=0,
                           allow_small_or_imprecise_dtypes=True)
            iota2b_sb = cpool.tile([128, BLK * 128], bf16, tag="iota2b")
            nc.gpsimd.iota(iota2b_sb[:], pattern=[[1, BLK * 128]], base=0,
                           channel_multiplier=0,
                           allow_small_or_imprecise_dtypes=True)
            iota_p = cpool.tile([128, 1], f32, tag="iotap")
            nc.gpsimd.iota(iota_p[:], pattern=[[0, 1]], base=0,
                           channel_multiplier=1,
                           allow_small_or_imprecise_dtypes=True)
            iota_pk = [iota_p]
            for kk in range(1, BLK):
                t = cpool.tile([128, 1], f32, tag=f"iotap{kk}")
                nc.vector.tensor_scalar_add(t[:], iota_p[:],
                                            float(kk * 128))
                iota_pk.append(t)
            ident_sb = cpool.tile([128, 128], f32, tag="ident")
            nc.vector.tensor_scalar(out=ident_sb[:],
                                    in0=iota2_sb[:, 0:128],
                                    scalar1=iota_p[:], scalar2=None,
                                    op0=OP.is_equal)
            er1_sb = cpool.tile([128, nchunk, heads], bf16, tag="er1")
            er2_sb = cpool.tile([128, nchunk, 1], bf16, tag="er2")

            # ---------------- phase A (sharded; AllGather h in halves) -----
            def ag(shard, tab):
                nc.gpsimd.collective_compute(
                    "AllGather", OP.bypass, replica_groups=replica,
                    ins=[shard.opt()], outs=[tab.opt()])

            def phase_a(actx):
                apool = actx.enter_context(tc.tile_pool(name="phA", bufs=2))
                apsum = actx.enter_context(
                    tc.tile_pool(name="phAps", bufs=4, space="PSUM"))
                t_starts = []
                for s0, s1 in ((0, hchunk), (hchunk, ntile_a)):
                    t_starts += [(t0, min(FB, s1 - t0))
                                 for t0 in range(s0, s1, FB)]
                for (bt0, jn) in t_starts:
                    ft = apool.tile([128, kt, FB * 128], bf16, tag="ft")
                    for k in range(kt):
                        nc.sync.dma_start(
                            ft[:, k, 0:jn * 128],
                            featT[k * 128:(k + 1) * 128,
                                  bt0 * 128:bt0 * 128 + jn * 128])
                    rowb = apool.tile([128, FB, ROW1], bf16, tag="rowb")
                    for j in range(jn):
                        t = bt0 + j
                        ps = apsum.tile([128, hw1], f32, tag="hps")
                        for k in range(kt):
                            nc.tensor.matmul(
                                ps[:], lhsT=ft[:, k, j * 128:(j + 1) * 128],
                                rhs=w1_sb[:, k * hw1:(k + 1) * hw1],
                                start=(k == 0), stop=(k == kt - 1))
                        nc.scalar.copy(rowb[:, j, 0:hd], ps[:, 0:hd])
                        nc.scalar.copy(er1_sb[:, t, :],
                                       ps[:, hd:hw1])
                    r0 = bt0 * 128
                    if r0 < hszA:
                        dst = hshardA[r0:r0 + jn * 128, :]
                    else:
                        dst = hshardB[r0 - hszA:r0 - hszA + jn * 128, :]
                    nc.sync.dma_start(
                        dst.rearrange("(s p) r -> p s r", p=128),
                        rowb[:, 0:jn, :])
                    if rowsA and r0 + jn * 128 == hszA:
                        ag(hshardA, htabA)
                ag(hshardB, htabB)

            # ---------------- edge phases ----------------
            def post_chunk_l1(k, ps, ppost, pps2):
                fw, sw = hd, heads
                den = ppost.tile([128, sw], f32, tag="den")
                nc.vector.tensor_scalar_max(den[:], ps[:, fw:fw + sw], 1e-30)
                rec = ppost.tile([128, sw], f32, tag="rec")
                nc.vector.reciprocal_approx_fast(rec[:], den[:])
                h2 = ppost.tile([128, fw], f32, tag="h2")
                nc.vector.tensor_mul(
                    h2[:].rearrange("p (d s) -> p d s", s=sw),
                    ps[:, 0:fw].rearrange("p (d s) -> p d s", s=sw),
                    rec[:].unsqueeze(1).broadcast_to([128, fw // sw, sw]))
                nc.vector.tensor_add(h2[:], h2[:], b1_sb[:])
                mn = ppost.tile([128, fw], f32, tag="mn")
                nc.vector.tensor_scalar_min(mn[:], h2[:], 0.0)
                nc.scalar.activation(mn[:], mn[:], AF.Exp)
                nc.vector.scalar_tensor_tensor(
                    out=h2[:], in0=h2[:], scalar=0.0,
                    in1=mn[:], op0=OP.max, op1=OP.add)
                nc.vector.tensor_scalar_sub(h2[:], h2[:], 1.0)
                # L2 node phase
                pst = pps2.tile([128, 128], f32, tag="pst")
                nc.tensor.transpose(pst[:], h2[:], ident_sb[:])
                h2T = ppost.tile([128, 128], f32, tag="h2T")
                nc.scalar.copy(h2T[:], pst[:])
                ps2 = pps2.tile([128, n_classes + 2], f32, tag="hh")
                nc.tensor.matmul(ps2[:], lhsT=h2T[:], rhs=w2_sb[:],
                                 start=True, stop=True)
                # l2 row = [hh(16) | 1.0 | el2 | garbage-pad]; the ones
                # column lets the Bex aggregation produce the softmax
                # denominator for free.
                l2r = ppost.tile([128, ROW2], bf16, tag="l2r")
                nc.scalar.copy(l2r[:, 0:n_classes], ps2[:, 0:n_classes])
                nc.vector.memset(l2r[:, n_classes:n_classes + 1], 1.0)
                nc.scalar.copy(l2r[:, n_classes + 1:n_classes + 2],
                               ps2[:, n_classes:n_classes + 1])
                nc.scalar.copy(er2_sb[:, k, :],
                               ps2[:, n_classes + 1:n_classes + 2])
                if k < hchunk:
                    nc.sync.dma_start(l2shardA[k * CH:(k + 1) * CH, :],
                                      l2r[:])
                else:
                    kb = k - hchunk
                    nc.sync.dma_start(l2shardB[kb * CH:(kb + 1) * CH, :],
                                      l2r[:])

            def post_chunk_l2(k, ps, ppost):
                fw = n_classes
                den = ppost.tile([128, 1], f32, tag="den2")
                nc.vector.tensor_scalar_max(den[:], ps[:, fw:fw + 1], 1e-30)
                rec = ppost.tile([128, 1], f32, tag="rec2")
                nc.vector.reciprocal_approx_fast(rec[:], den[:])
                xx = ppost.tile([128, fw], f32, tag="xx")
                nc.vector.tensor_scalar(out=xx[:], in0=ps[:, 0:fw],
                                        scalar1=rec[:], scalar2=None,
                                        op0=OP.mult)
                nc.vector.tensor_add(xx[:], xx[:], b2_sb[:])
                rmax = ppost.tile([128, 1], f32, tag="rmax")
                nc.vector.tensor_reduce(out=rmax[:], in_=xx[:],
                                        axis=AX.X, op=OP.max)
                nc.vector.tensor_scalar(out=xx[:], in0=xx[:],
                                        scalar1=rmax[:], scalar2=None,
                                        op0=OP.subtract)
                exs = ppost.tile([128, fw], f32, tag="exs")
                ssum = ppost.tile([128, 1], f32, tag="ssum")
                nc.scalar.activation(exs[:], xx[:], AF.Exp,
                                     accum_out=ssum[:])
                lss = ppost.tile([128, 1], f32, tag="lss")
                nc.scalar.activation(lss[:], ssum[:], AF.Ln)
                nc.vector.tensor_scalar(out=xx[:], in0=xx[:],
                                        scalar1=lss[:], scalar2=None,
                                        op0=OP.subtract)
                nc.sync.dma_start(out_d[k * CH:(k + 1) * CH, :], xx[:])

            def edge_phase(layer, mid_blk=None, mid_cb=None):
                lm = meta["el"]
                if layer == 1:
                    rw, fw, sw, gdt = ROW1, hd, heads, bf16
                    er_sb, twin = er1_sb, hwin
                else:
                    rw, fw, sw, gdt = ROW2, n_classes, 1, bf16
                    er_sb, twin = er2_sb, l2win
                nw = fw + sw
                nblk = lm["nblk"]
                gath, blocks = lm["gath"], lm["blocks"]
                er_ks, agg_tiles = lm["er_ks"], lm["agg_tiles"]
                bplanes = lm["bplanes"]

                gblocks = {}
                for (b, w, g0, nt) in gath:
                    gblocks.setdefault(b, []).append((w, g0, nt))

                with ExitStack() as ectx:
                    pool = ectx.enter_context(
                        tc.tile_pool(name=f"edge{layer}", bufs=2))
                    pps = ectx.enter_context(
                        tc.tile_pool(name=f"eps{layer}", bufs=2,
                                     space="PSUM"))
                    ppost = ectx.enter_context(
                        tc.tile_pool(name=f"post{layer}", bufs=2))
                    pps2 = ectx.enter_context(
                        tc.tile_pool(name=f"ep2{layer}", bufs=2,
                                     space="PSUM"))
                    ppsE = ectx.enter_context(
                        tc.tile_pool(name=f"epE{layer}", bufs=2,
                                     space="PSUM"))
                    for b in range(nblk):
                        t0, tb = blocks[b]
                        assert tb * sw <= 512, (tb, sw)
                        segs = gblocks[b]
                        gt = pool.tile([128, tb, rw], gdt, tag="gt")
                        ii = pool.tile([128, tb * 8], i16, tag="gi")
                        nc.sync.dma_start(
                            ii[:], gidx_d[:, t0 * 8:(t0 + tb) * 8])
                        if bulk:
                            tabsrc = htabB if layer == 1 else l2tabB
                            nc.sync.dma_start(
                                gt[:],
                                tabsrc[0:128 * tb, :].rearrange(
                                    "(p s) r -> p s r", p=128))
                        for (w, g0, nt) in segs:
                            if bulk:
                                continue
                            t_, lo, hi = twin[w]
                            tab = t_[lo:hi, :]
                            for s0 in range(0, nt, GMAX):
                                sn = min(GMAX, nt - s0)
                                a = g0 - t0 + s0
                                nc.gpsimd.dma_gather(
                                    out_ap=gt[:, a:a + sn, :],
                                    in_ap=tab,
                                    idxs_ap=ii[:, a * 8:(a + sn) * 8],
                                    num_idxs=sn * 128,
                                    num_idxs_reg=sn * 128, elem_size=rw,
                                    queue_num=next_q())
                        # one-hot builds (all-bf16 SBUF -> DVE 4x mode)
                        slF = pool.tile([128, tb * 128], bf16, tag="slF")
                        nc.sync.dma_start(
                            slF[:],
                            slotF_d[0:1, t0 * 128:(t0 + tb) * 128]
                            .broadcast_to([128, tb * 128]))
                        BT = pool.tile([128, BLK, tb * 128], bf16, tag="BT")
                        sl = pool.tile([128, tb], f32, tag="sl")
                        nc.sync.dma_start(sl[:], slotB_d[:, t0:t0 + tb])
                        for kk in range(BLK):
                            rng = lm["plane_rng"][b][kk]
                            if rng is None:
                                continue
                            lo, hi = rng
                            nc.vector.tensor_scalar(
                                out=BT[:, kk, lo * 128:hi * 128],
                                in0=slF[:, lo * 128:hi * 128],
                                scalar1=iota_pk[kk][:], scalar2=None,
                                op0=OP.is_equal)
                        # per-edge er via transposed one-hot matmul
                        psE = ppsE.tile([128, tb * sw], f32, tag="psE")
                        for t in range(tb):
                            ks = er_ks[b][t]
                            for i, kk in enumerate(ks):
                                k = b * BLK + kk
                                nc.tensor.matmul(
                                    psE[:, t * sw:(t + 1) * sw],
                                    lhsT=BT[:, kk, t * 128:(t + 1) * 128],
                                    rhs=er_sb[:, k, :],
                                    start=(i == 0), stop=(i == len(ks) - 1))
                        B = pool.tile([128, tb, BLK * 128], bf16, tag="B")
                        ex = pool.tile([128, tb, sw], f32, tag="ex")
                        if layer == 1:
                            # el recompute: tmp = gt_h * al ((d s) bf16, 2x)
                            # then tree-fold over d down to [tb, sw]
                            tmp = pool.tile([128, tb, hd], bf16, tag="tmpel")
                            nc.vector.tensor_mul(
                                tmp[:], gt[:, :, 0:hd],
                                al_sb[:].unsqueeze(1)
                                .broadcast_to([128, tb, hd]))
                            for half in (hd // 2, hd // 4, hd // 8):
                                nc.vector.tensor_add(
                                    tmp[:, :, 0:half], tmp[:, :, 0:half],
                                    tmp[:, :, half:2 * half])
                            nc.vector.tensor_add(
                                ex[:], tmp[:, :, 0:sw],
                                tmp[:, :, sw:2 * sw])
                            nc.vector.tensor_add(
                                ex[:], ex[:],
                                psE[:].rearrange("p (t s) -> p t s", s=sw))
                        else:
                            nc.vector.tensor_add(
                                ex[:], gt[:, :, fw + 1:fw + 2],
                                psE[:].rearrange("p (t s) -> p t s", s=sw))
                        nc.vector.scalar_tensor_tensor(
                            out=ex[:], in0=ex[:], scalar=NEG_SLOPE,
                            in1=ex[:], op0=OP.mult, op1=OP.max)
                        if layer == 1:
                            comb = pool.tile([128, tb, nw], bf16, tag="comb")
                            nc.scalar.activation(comb[:, :, fw:fw + sw],
                                                 ex[:], AF.Exp)
                            for (kk, t) in bplanes[b]:
                                nc.vector.tensor_scalar(
                                    out=B[:, t, kk * 128:(kk + 1) * 128],
                                    in0=iota2b_sb[:, kk * 128:(kk + 1) * 128],
                                    scalar1=sl[:, t:t + 1], scalar2=None,
                                    op0=OP.is_equal)
                            nc.vector.tensor_mul(
                                comb[:, :, 0:fw].rearrange(
                                    "p t (d s) -> p t d s", s=sw),
                                gt[:, :, 0:fw].rearrange(
                                    "p t (d s) -> p t d s", s=sw),
                                comb[:, :, fw:fw + sw].unsqueeze(2)
                                .broadcast_to([128, tb, fw // sw, sw]))
                        else:
                            # fold exp(ex) into the one-hot: Bex = onehot*ex;
                            # rhs = [hh | 1] straight from the gathered rows
                            nc.scalar.activation(ex[:], ex[:], AF.Exp)
                            for (kk, t) in bplanes[b]:
                                nc.vector.tensor_scalar(
                                    out=B[:, t, kk * 128:(kk + 1) * 128],
                                    in0=iota2b_sb[:, kk * 128:(kk + 1) * 128],
                                    scalar1=sl[:, t:t + 1],
                                    scalar2=ex[:, t, :],
                                    op0=OP.is_equal, op1=OP.mult)
                        for kk in range(BLK):
                            k = b * BLK + kk
                            if k >= nchunk:
                                break
                            tl = agg_tiles[b][kk]
                            ps = pps.tile([128, nw], f32, tag="agg")
                            for j, t in enumerate(tl):
                                rhs = (comb[:, t - t0, :] if layer == 1
                                       else gt[:, t - t0, 0:nw])
                                nc.tensor.matmul(
                                    ps[:], lhsT=B[:, t - t0,
                                                  kk * 128:(kk + 1) * 128],
                                    rhs=rhs,
                                    start=(j == 0), stop=(j == len(tl) - 1))
                            if layer == 1:
                                post_chunk_l1(k, ps, ppost, pps2)
                            else:
                                post_chunk_l2(k, ps, ppost)
                        if mid_blk is not None and b == mid_blk:
                            mid_cb()

            for _rep in range(krep):
                with ExitStack() as actx:
                    phase_a(actx)
                if kphase != "A":
                    if rowsA:
                        edge_phase(1, mid_blk=hchunk // BLK - 1,
                                   mid_cb=lambda: ag(l2shardA, l2tabA))
                    else:
                        edge_phase(1)
                    ag(l2shardB, l2tabB)
                    if kphase != "L1":
                        edge_phase(2)

    nc_.compile()
    return nc_


# ------------------------------------------------------------------ driver
def make_in_maps(meta, feat, W1, al1, ar1, b1, W2, al2, ar2, b2):
    import ml_dtypes
    bf16 = ml_dtypes.bfloat16
    sh, n_pad = meta["sh"], meta["n_pad"]
    n = meta["n_nodes"]
    feat_pad = np.zeros((n_pad, feat.shape[1]), np.float32)
    feat_pad[:n] = feat
    # fused [W1 | W1@ar]: er is linear in feat.  Hidden columns are stored
    # head-INTERLEAVED ((d s): col = d*heads + s) so per-head broadcasts in
    # the edge phase have a packed last dim (DVE 2x perf mode).
    heads, hid = al1.shape
    # new[:, d*H+s] = old[:, s*hid+d]:  new = old[:, inv] with inv[d*H+s]=s*hid+d
    inv = (np.arange(heads)[None, :] * hid
           + np.arange(hid)[:, None]).reshape(-1)
    w1r = W1.reshape(-1, heads, hid)
    w1ar = np.einsum("fhd,hd->fh", w1r, ar1)
    W1a = np.concatenate([W1[:, inv], w1ar], axis=1).astype(np.float32)
    # al in (d s) order: al_ds[d*H+s] = al1[s, d]
    alrep = np.tile(al1.T.reshape(1, -1), (128, 1)).astype(bf16)
    b1rep = np.tile(b1[inv].reshape(1, -1), (128, 1)).astype(np.float32)
    W2a = np.concatenate(
        [W2, W2 @ al2.reshape(-1, 1), W2 @ ar2.reshape(-1, 1)],
        axis=1).astype(np.float32)[inv, :]
    b2rep = np.tile(b2.reshape(1, -1), (128, 1)).astype(np.float32)
    el = meta["el"]
    in_maps = []
    for c in range(NC):
        in_maps.append({
            "featT": np.ascontiguousarray(
                feat_pad[c * sh:(c + 1) * sh].T).astype(bf16),
            "W1": W1a.astype(bf16),
            "b1rep": b1rep, "alrep": alrep,
            "W2a": W2a, "b2rep": b2rep,
            "gidx": _wrap16(el["gidx"][c]),
            "slotB": np.ascontiguousarray(
                el["slotB"][c].reshape(-1, 128).T).astype(np.float32),
            "slotF": np.maximum(el["slotB"][c], 0.0).reshape(1, -1)
            .astype(bf16),
        })
    return in_maps


class Runner:
    """Builds the SPMD program once; exposes a repeatable timed executor."""

    def __init__(self, meta, f_in):
        self.meta = meta
        self.nc = build_program(meta, f_in, HID, HEADS, N_CLASSES)
        self._fn = None

    def _lower(self):
        import jax
        import numpy as _np
        from jax.sharding import Mesh, PartitionSpec
        from jax.experimental.shard_map import shard_map
        from concourse import mybir
        from concourse.bass2jax import _bass_exec_p, install_neuronx_cc_hook

        install_neuronx_cc_hook()
        nc = self.nc
        in_names, out_names, out_avals, zero_outs = [], [], [], []
        partition_name = (nc.partition_id_tensor.name
                          if nc.partition_id_tensor else None)
        for alloc in nc.m.functions[0].allocations:
            if not isinstance(alloc, mybir.MemoryLocationSet):
                continue
            name = alloc.memorylocations[0].name
            if alloc.kind == "ExternalInput":
                if name != partition_name:
                    in_names.append(name)
            elif alloc.kind == "ExternalOutput":
                shape = tuple(alloc.tensor_shape)
                dtype = mybir.dt.np(alloc.dtype)
                out_names.append(name)
                out_avals.append(jax.core.ShapedArray(shape, dtype))
                zero_outs.append(_np.zeros(shape, dtype))
        n_params = len(in_names)
        n_outs = len(out_avals)
        all_in_names = list(in_names) + list(out_names)
        if partition_name is not None:
            all_in_names.append(partition_name)

        def _body(*args):
            ins = list(args[:n_params])
            zouts = list(args[n_params:])
            operands = ins + zouts
            if partition_name is not None:
                from concourse.bass2jax import partition_id_tensor
                operands.append(partition_id_tensor())
            outs = _bass_exec_p.bind(
                *operands, out_avals=tuple(out_avals),
                in_names=tuple(all_in_names), out_names=tuple(out_names),
                lowering_input_output_aliases=(),
                sim_require_finite=False, sim_require_nnan=False, nc=nc)
            return tuple(outs)

        devices = jax.devices()[:NC]
        mesh = Mesh(_np.asarray(devices), ("core",))
        in_specs = (PartitionSpec("core"),) * (n_params + n_outs)
        out_specs = (PartitionSpec("core"),) * n_outs
        self._fn = jax.jit(
            shard_map(_body, mesh=mesh, in_specs=in_specs,
                      out_specs=out_specs, check_rep=False),
            keep_unused=True)
        self._in_names = in_names
        self._out_names = out_names
        self._out_avals = out_avals
        self._zero_outs = zero_outs
        self._mesh = mesh
        self._in_specs = in_specs

    def prepare(self, in_maps):
        import jax
        import numpy as _np
        from jax.sharding import NamedSharding, PartitionSpec
        if self._fn is None:
            self._lower()
        concat_in = [
            _np.concatenate([in_maps[c][name] for c in range(NC)], axis=0)
            for name in self._in_names]
        concat_zeros = [
            _np.zeros((NC * z.shape[0], *z.shape[1:]), z.dtype)
            for z in self._zero_outs]
        shd = NamedSharding(self._mesh, PartitionSpec("core"))
        self._args = [jax.device_put(a, shd) for a in concat_in + concat_zeros]
        jax.block_until_ready(self._args)

    def run(self):
        import jax
        out = self._fn(*self._args)
        out = jax.block_until_ready(out)
        import numpy as _np
        res = _np.asarray(out[self._out_names.index("out")])
        shp = self._out_avals[self._out_names.index("out")].shape
        res = res.reshape(NC, *shp)
        n, sh = self.meta["n_nodes"], self.meta["sh"]
        parts = [res[c][:min(sh, max(0, n - c * sh))] for c in range(NC)]
        return _np.concatenate(parts, axis=0)


_RUNNER = None


def get_runner(feat, src, dst):
    global _RUNNER
    n, f_in = feat.shape
    meta = host_prep(np.asarray(src, np.int32), np.asarray(dst, np.int32),
                     n_nodes=n)
    _RUNNER = Runner(meta, f_in)
    return _RUNNER


def kernel(feat, src, dst, W1, al1, ar1, b1, W2, al2, ar2, b2):
    feat = np.asarray(feat, dtype=np.float32)
    src = np.asarray(src, dtype=np.int32)
    dst = np.asarray(dst, dtype=np.int32)
    args = [np.asarray(x, np.float32)
            for x in (W1, al1, ar1, b1, W2, al2, ar2, b2)]
    r = _RUNNER if _RUNNER is not None else get_runner(feat, src, dst)
    in_maps = make_in_maps(r.meta, feat, *args)
    r.prepare(in_maps)
    return r.run()


kernel.last_exec_time_ns = None



# revision 19
# speedup vs baseline: 2.5167x; 1.3049x over previous
"""Bass/Trainium2 kernel for a 2-layer GAT (GATConv x2 + log_softmax) on 8 NeuronCores.

Strategy (edge/data parallel, dst-sharded, v2):
  - Nodes sharded 8 ways by id (padded shard sh = 128*ceil(N/1024)); core c
    owns dst nodes [c*sh, (c+1)*sh).
  - Phase A is REPLICATED: every core computes h|el for ALL nodes into its
    own local htab (no collective).  Each core's htab/featT are ROTATED by
    c*sh so the first nchunk tiles are its own dst shard -- er (attention
    right-term) for local chunks is kept in SBUF, never in HBM.
  - L1 edge phase (edges partitioned by dst owner, sorted by (block of 2
    chunks, src window, chunk)): per 128-edge tile, dma_gather h[src] rows
    (512B) from htab windows; per-edge er comes from a transposed one-hot
    PE matmul BT[slot,e] @ er_chunk (no er gather); aggregate messages with
    PE matmuls psum[slot,:] += B^T @ [exp(leaky(el+er)) * h[src] | exp(..)].
  - L2 node work fused per chunk: hh|el2|er2 = h2T @ [W2|W2@al2|W2@ar2];
    hh|el2 rows -> l2shard, AllGather -> l2tab (global node order); er2
    stays in SBUF.  L2 edge phase repeats the pipeline on 256B rows, then
    log_softmax into the output shard.
  - Gathers use 4 SWDGE queues round-robin; gather idx are int16 against
    32768-row windows (htab is split into per-window tensors so gathers can
    start before all of phase A finishes).
"""

import os
import sys

import numpy as np

sys.path.insert(0, "/opt/trn_rl_repo")

# ---------------------------------------------------------------- constants
N_NODES = 100000
F_IN = 256
HID = 16
HEADS = 8
N_CLASSES = 16
NEG_SLOPE = 0.2
NC = 8                      # cores
CH = 128                    # dst nodes per chunk
BLK = 2                     # chunks per block (edges padded per (block,win))
GMAX = 8                    # max 128-edge tiles per dma_gather instruction
ROW1 = 128                  # bf16 per L1 table row (h only; el recomputed)
ROW2 = 128                  # bf16 per L2 table row (hh 16 | el2 1 | pad)
WIN = 32768                 # rows addressable by one int16 gather window
FB = 8                      # phase-A node tiles per DMA batch


def _wrap16(v):
    # [n] -> [128, n//16] int16; idx i at [i%16, i//16], replicated over groups
    n = v.shape[0]
    assert n % 16 == 0
    a = v.reshape(n // 16, 16).T.astype(np.int16)      # [16, n//16]
    return np.ascontiguousarray(np.tile(a, (8, 1)))    # [128, n//16]


def _prep_layer(src, dst, n_nodes, sh, n_pad, nchunk, keyfn, win_bounds):
    """Edge layout for one layer.  keyfn maps global src id -> table row;
    win_bounds (ascending, last == n_pad) defines gather windows."""
    wb = np.asarray(win_bounds, np.int64)
    nwin = len(wb) - 1
    assert (wb[1:] - wb[:-1] <= WIN).all()
    nblk = (nchunk + BLK - 1) // BLK

    per_core = []
    cnt = np.zeros((NC, nblk, nwin), dtype=np.int64)
    for c in range(NC):
        m = (dst >= c * sh) & (dst < (c + 1) * sh)
        es = src[m].astype(np.int64)
        ed = (dst[m] - c * sh).astype(np.int64)
        key = keyfn(es, c)
        chunk = ed // CH
        block = chunk // BLK
        win = np.searchsorted(wb, key, side="right") - 1
        order = np.lexsort((key, chunk, win, block))
        es, ed, key, chunk, block, win = (
            x[order] for x in (es, ed, key, chunk, block, win))
        per_core.append((key, ed, chunk, block, win))
        np.add.at(cnt[c], (block, win), 1)

    T = np.ceil(cnt.max(axis=0) / float(CH)).astype(np.int64)  # [nblk, nwin]
    for b in range(nblk):
        if T[b].sum() == 0:
            T[b, 0] = 1

    seg_off = np.zeros((nblk, nwin), dtype=np.int64)
    gath = []                 # (block, window, tile_off, ntiles)
    blocks = []               # (t0, tb) per block
    toff = 0
    for b in range(nblk):
        t0 = toff
        for w in range(nwin):
            seg_off[b, w] = toff
            if T[b, w]:
                gath.append((b, w, toff, int(T[b, w])))
            toff += int(T[b, w])
        blocks.append((t0, toff - t0))
    ntile = toff
    ne_pad = ntile * CH

    gidx = np.zeros((NC, ne_pad), dtype=np.int16)
    slotB = np.full((NC, ne_pad), -1.0, dtype=np.float32)
    flags = np.zeros((ntile, BLK), dtype=bool)
    for c in range(NC):
        key, ed, chunk, block, win = per_core[c]
        # edges are sorted by (block, win, chunk); place each (b,w) group at
        # its segment offset
        bw = block * nwin + win
        grp_start = np.searchsorted(bw, np.arange(nblk * nwin), side="left")
        grp_end = np.searchsorted(bw, np.arange(nblk * nwin), side="right")
        for b in range(nblk):
            for w in range(nwin):
                a, e = int(grp_start[b * nwin + w]), int(grp_end[b * nwin + w])
                if e == a:
                    continue
                pos = seg_off[b, w] * CH
                n = e - a
                gidx[c, pos:pos + n] = (key[a:e] - wb[win[a:e]]).astype(
                    np.int16)
                kk = chunk[a:e] - b * BLK
                slotB[c, pos:pos + n] = (kk * CH + ed[a:e] % CH).astype(
                    np.float32)
                tl = (pos + np.arange(n)) // CH
                flags[tl, kk] = True

    # per-block matmul emission lists (uniform across cores)
    er_ks = []                # [block][tile] -> list of kk with edges
    agg_tiles = []            # [block][kk] -> stream-ordered tiles
    plane_rng = []            # [block][kk] -> (lo, hi) local tile range
    bplanes = []              # [block][kk] -> (lo, hi) B plane tile range
    for b in range(nblk):
        t0, tb = blocks[b]
        ek = []
        for t in range(t0, t0 + tb):
            ks = [kk for kk in range(BLK) if flags[t, kk]]
            ek.append(ks if ks else [0])
        er_ks.append(ek)
        at = []
        pr = []
        bp = []
        for kk in range(BLK):
            tl = [t for t in range(t0, t0 + tb) if flags[t, kk]]
            at.append(tl if tl else [t0])
            need = [t - t0 for t in at[kk]]
            agg_rng = (min(need), max(need) + 1)
            need += [i for i, ks in enumerate(ek) if kk in ks]
            pr.append((min(need), max(need) + 1) if need else None)
            bp.append(agg_rng)
        agg_tiles.append(at)
        plane_rng.append(pr)
        bplanes.append(bp)

    return dict(
        nwin=nwin, nblk=nblk, ntile=ntile, ne_pad=ne_pad, gath=gath,
        blocks=blocks, er_ks=er_ks, agg_tiles=agg_tiles,
        plane_rng=plane_rng, bplanes=bplanes, gidx=gidx, slotB=slotB,
    )


def host_prep(src, dst, n_nodes=N_NODES, nc=NC, ch=CH, win_edge=None):
    src = np.asarray(src, np.int64)
    dst = np.asarray(dst, np.int64)
    nchunk = (n_nodes + CH * NC - 1) // (CH * NC)
    sh = nchunk * CH
    n_pad = NC * sh

    # Both layers' tables live in a half-split layout so each AllGather can
    # be issued in two pieces: half A = dst slots [0, hsz) of every core
    # (table rows [0, rowsA)), half B = the rest.
    hchunk = (nchunk // (2 * BLK)) * BLK
    hsz = hchunk * CH
    rowsA = NC * hsz

    def key2(es, c):
        co = es // sh
        r = es % sh
        inA = r < hsz
        return np.where(inA, co * hsz + r,
                        rowsA + co * (sh - hsz) + (r - hsz))

    wb2 = (list(range(0, rowsA, WIN)) + [rowsA] if rowsA else [0])
    wb2 += [w for w in range(rowsA + WIN, n_pad, WIN)] + [n_pad]
    wb2 = sorted(set(wb2))
    el = _prep_layer(src, dst, n_nodes, sh, n_pad, nchunk, key2, wb2)
    return dict(n_nodes=n_nodes, sh=sh, n_pad=n_pad, nchunk=nchunk,
                hchunk=hchunk, rowsA=rowsA, wb2=wb2, el=el)


# ------------------------------------------------------------- bass program
def build_program(meta, f_in, hid, heads, n_classes):
    from contextlib import ExitStack

    import concourse.tile as tile
    from concourse import bacc, mybir

    dt = mybir.dt
    f32, bf16, i16 = dt.float32, dt.bfloat16, dt.int16
    AF = mybir.ActivationFunctionType
    OP = mybir.AluOpType
    AX = mybir.AxisListType

    n_pad, sh, nchunk = meta["n_pad"], meta["sh"], meta["nchunk"]
    hd = heads * hid
    kt = f_in // 128
    ntile_a = sh // 128
    nwin = meta["el"]["nwin"]

    nq = int(os.environ.get("BASS_QUEUES", "4"))
    bulk = os.environ.get("BASS_BULK", "0") == "1"
    kphase = os.environ.get("BASS_KPHASE", "full")
    krep = int(os.environ.get("BASS_REPEAT", "1"))

    nc_ = bacc.Bacc("TRN2", target_bir_lowering=False, debug=False,
                    num_devices=NC, num_swdge_queues=nq)
    qctr = [0]

    def next_q():
        q = qctr[0] % nq
        qctr[0] += 1
        return q

    def din(name, shape, dtype):
        return nc_.dram_tensor(name, list(shape), dtype,
                               kind="ExternalInput").ap()

    hw1 = hd + heads                # h | er fused matmul width
    featT = din("featT", [f_in, sh], bf16)
    W1 = din("W1", [f_in, hw1], bf16)
    b1rep = din("b1rep", [128, hd], f32)
    alrep = din("alrep", [128, hd], bf16)
    W2a = din("W2a", [hd, n_classes + 2], f32)
    b2rep = din("b2rep", [128, n_classes], f32)
    gidx_d = din("gidx", [128, meta["el"]["ne_pad"] // 16], i16)
    slotB_d = din("slotB", [128, meta["el"]["ntile"]], f32)
    slotF_d = din("slotF", [1, meta["el"]["ne_pad"]], bf16)
    out_d = nc_.dram_tensor("out", [sh, n_classes], f32,
                            kind="ExternalOutput").ap()

    wb2 = meta["wb2"]
    hchunk, rowsA = meta["hchunk"], meta["rowsA"]
    rowsB = n_pad - rowsA
    hszA = hchunk * CH

    def mk_pair(name, row, dtype):
        tabA = (nc_.dram_tensor(f"{name}tabA", [rowsA, row], dtype,
                                addr_space="Shared").ap() if rowsA else None)
        tabB = nc_.dram_tensor(f"{name}tabB", [rowsB, row], dtype,
                               addr_space="Shared").ap()
        shA = (nc_.dram_tensor(f"{name}shardA", [hszA, row], dtype).ap()
               if rowsA else None)
        shB = nc_.dram_tensor(f"{name}shardB", [sh - hszA, row],
                              dtype).ap()
        twin = []
        for w in range(len(wb2) - 1):
            lo, hi = wb2[w], wb2[w + 1]
            if lo < rowsA:
                assert hi <= rowsA
                twin.append((tabA, lo, hi))
            else:
                twin.append((tabB, lo - rowsA, hi - rowsA))
        return tabA, tabB, shA, shB, twin

    htabA, htabB, hshardA, hshardB, hwin = mk_pair("h", ROW1, bf16)
    l2tabA, l2tabB, l2shardA, l2shardB, l2win = mk_pair("l2", ROW2, bf16)

    replica = [list(range(NC))]

    with tile.TileContext(nc_) as tc:
        nc = tc.nc
        with ExitStack() as cctx:
            cpool = cctx.enter_context(tc.tile_pool(name="const", bufs=1))
            w1_sb = cpool.tile([128, kt * hw1], bf16, tag="w1")
            for k in range(kt):
                nc.sync.dma_start(w1_sb[:, k * hw1:(k + 1) * hw1],
                                  W1[k * 128:(k + 1) * 128, :])
            al_sb = cpool.tile([128, hd], bf16, tag="al")
            nc.sync.dma_start(al_sb[:], alrep[:])
            b1_sb = cpool.tile([128, hd], f32, tag="b1")
            nc.sync.dma_start(b1_sb[:], b1rep[:])
            w2_sb = cpool.tile([hd, n_classes + 2], f32, tag="w2")
            nc.sync.dma_start(w2_sb[:], W2a[:])
            b2_sb = cpool.tile([128, n_classes], f32, tag="b2")
            nc.sync.dma_start(b2_sb[:], b2rep[:])
            iota2_sb = cpool.tile([128, 128], f32, tag="iota2")
            nc.gpsimd.iota(iota2_sb[:], pattern=[[1, 128]], base=0,
                           channel_multiplier=0,
                           allow_small_or_imprecise_dtypes=True)
            iota2b_sb = cpool.tile([128, BLK * 128], bf16, tag="iota2b")
            nc.gpsimd.iota(iota2b_sb[:], pattern=[[1, BLK * 128]], base=0,
                           channel_multiplier=0,
                           allow_small_or_imprecise_dtypes=True)
            iota_p = cpool.tile([128, 1], f32, tag="iotap")
            nc.gpsimd.iota(iota_p[:], pattern=[[0, 1]], base=0,
                           channel_multiplier=1,
                           allow_small_or_imprecise_dtypes=True)
            iota_pk = [iota_p]
            for kk in range(1, BLK):
                t = cpool.tile([128, 1], f32, tag=f"iotap{kk}")
                nc.vector.tensor_scalar_add(t[:], iota_p[:],
                                            float(kk * 128))
                iota_pk.append(t)
            ident_sb = cpool.tile([128, 128], f32, tag="ident")
            nc.vector.tensor_scalar(out=ident_sb[:],
                                    in0=iota2_sb[:, 0:128],
                                    scalar1=iota_p[:], scalar2=None,
                                    op0=OP.is_equal)
            er1_sb = cpool.tile([128, nchunk, heads], bf16, tag="er1")
            er2_sb = cpool.tile([128, nchunk, 1], bf16, tag="er2")

            # ---------------- phase A (sharded; AllGather h in halves) -----
            def ag(shard, tab):
                nc.gpsimd.collective_compute(
                    "AllGather", OP.bypass, replica_groups=replica,
                    ins=[shard.opt()], outs=[tab.opt()])

            def phase_a(actx):
                apool = actx.enter_context(tc.tile_pool(name="phA", bufs=2))
                apsum = actx.enter_context(
                    tc.tile_pool(name="phAps", bufs=4, space="PSUM"))
                t_starts = []
                for s0, s1 in ((0, hchunk), (hchunk, ntile_a)):
                    t_starts += [(t0, min(FB, s1 - t0))
                                 for t0 in range(s0, s1, FB)]
                for (bt0, jn) in t_starts:
                    ft = apool.tile([128, kt, FB * 128], bf16, tag="ft")
                    for k in range(kt):
                        nc.sync.dma_start(
                            ft[:, k, 0:jn * 128],
                            featT[k * 128:(k + 1) * 128,
                                  bt0 * 128:bt0 * 128 + jn * 128])
                    rowb = apool.tile([128, FB, ROW1], bf16, tag="rowb")
                    for j in range(jn):
                        t = bt0 + j
                        ps = apsum.tile([128, hw1], f32, tag="hps")
                        for k in range(kt):
                            nc.tensor.matmul(
                                ps[:], lhsT=ft[:, k, j * 128:(j + 1) * 128],
                                rhs=w1_sb[:, k * hw1:(k + 1) * hw1],
                                start=(k == 0), stop=(k == kt - 1))
                        nc.scalar.copy(rowb[:, j, 0:hd], ps[:, 0:hd])
                        nc.scalar.copy(er1_sb[:, t, :],
                                       ps[:, hd:hw1])
                    r0 = bt0 * 128
                    if r0 < hszA:
                        dst = hshardA[r0:r0 + jn * 128, :]
                    else:
                        dst = hshardB[r0 - hszA:r0 - hszA + jn * 128, :]
                    nc.sync.dma_start(
                        dst.rearrange("(s p) r -> p s r", p=128),
                        rowb[:, 0:jn, :])
                    if rowsA and r0 + jn * 128 == hszA:
                        ag(hshardA, htabA)
                ag(hshardB, htabB)

            # ---------------- edge phases ----------------
            def post_chunk_l1(k, ps, ppost, pps2):
                fw, sw = hd, heads
                den = ppost.tile([128, sw], f32, tag="den")
                nc.vector.tensor_scalar_max(den[:], ps[:, fw:fw + sw], 1e-30)
                rec = ppost.tile([128, sw], f32, tag="rec")
                nc.vector.reciprocal_approx_fast(rec[:], den[:])
                h2 = ppost.tile([128, fw], f32, tag="h2")
                nc.vector.tensor_mul(
                    h2[:].rearrange("p (d s) -> p d s", s=sw),
                    ps[:, 0:fw].rearrange("p (d s) -> p d s", s=sw),
                    rec[:].unsqueeze(1).broadcast_to([128, fw // sw, sw]))
                nc.vector.tensor_add(h2[:], h2[:], b1_sb[:])
                mn = ppost.tile([128, fw], f32, tag="mn")
                nc.vector.tensor_scalar_min(mn[:], h2[:], 0.0)
                nc.scalar.activation(mn[:], mn[:], AF.Exp)
                nc.vector.scalar_tensor_tensor(
                    out=h2[:], in0=h2[:], scalar=0.0,
                    in1=mn[:], op0=OP.max, op1=OP.add)
                nc.vector.tensor_scalar_sub(h2[:], h2[:], 1.0)
                # L2 node phase
                pst = pps2.tile([128, 128], f32, tag="pst")
                nc.tensor.transpose(pst[:], h2[:], ident_sb[:])
                h2T = ppost.tile([128, 128], f32, tag="h2T")
                nc.scalar.copy(h2T[:], pst[:])
                ps2 = pps2.tile([128, n_classes + 2], f32, tag="hh")
                nc.tensor.matmul(ps2[:], lhsT=h2T[:], rhs=w2_sb[:],
                                 start=True, stop=True)
                # l2 row = [hh(16) | 1.0 | el2 | garbage-pad]; the ones
                # column lets the Bex aggregation produce the softmax
                # denominator for free.
                l2r = ppost.tile([128, ROW2], bf16, tag="l2r")
                nc.scalar.copy(l2r[:, 0:n_classes], ps2[:, 0:n_classes])
                nc.vector.memset(l2r[:, n_classes:n_classes + 1], 1.0)
                nc.scalar.copy(l2r[:, n_classes + 1:n_classes + 2],
                               ps2[:, n_classes:n_classes + 1])
                nc.scalar.copy(er2_sb[:, k, :],
                               ps2[:, n_classes + 1:n_classes + 2])
                if k < hchunk:
                    nc.sync.dma_start(l2shardA[k * CH:(k + 1) * CH, :],
                                      l2r[:])
                else:
                    kb = k - hchunk
                    nc.sync.dma_start(l2shardB[kb * CH:(kb + 1) * CH, :],
                                      l2r[:])

            def post_chunk_l2(k, ps, ppost):
                fw = n_classes
                den = ppost.tile([128, 1], f32, tag="den2")
                nc.vector.tensor_scalar_max(den[:], ps[:, fw:fw + 1], 1e-30)
                rec = ppost.tile([128, 1], f32, tag="rec2")
                nc.vector.reciprocal_approx_fast(rec[:], den[:])
                xx = ppost.tile([128, fw], f32, tag="xx")
                nc.vector.tensor_scalar(out=xx[:], in0=ps[:, 0:fw],
                                        scalar1=rec[:], scalar2=None,
                                        op0=OP.mult)
                nc.vector.tensor_add(xx[:], xx[:], b2_sb[:])
                rmax = ppost.tile([128, 8], f32, tag="rmax")
                nc.vector.max(rmax[:], xx[:])
                nc.vector.tensor_scalar(out=xx[:], in0=xx[:],
                                        scalar1=rmax[:, 0:1], scalar2=None,
                                        op0=OP.subtract)
                exs = ppost.tile([128, fw], f32, tag="exs")
                ssum = ppost.tile([128, 1], f32, tag="ssum")
                nc.scalar.activation(exs[:], xx[:], AF.Exp,
                                     accum_out=ssum[:])
                lss = ppost.tile([128, 1], f32, tag="lss")
                nc.scalar.activation(lss[:], ssum[:], AF.Ln)
                nc.vector.tensor_scalar(out=xx[:], in0=xx[:],
                                        scalar1=lss[:], scalar2=None,
                                        op0=OP.subtract)
                nc.sync.dma_start(out_d[k * CH:(k + 1) * CH, :], xx[:])

            def edge_phase(layer, mid_blk=None, mid_cb=None):
                lm = meta["el"]
                if layer == 1:
                    rw, fw, sw, gdt = ROW1, hd, heads, bf16
                    er_sb, twin = er1_sb, hwin
                else:
                    rw, fw, sw, gdt = ROW2, n_classes, 1, bf16
                    er_sb, twin = er2_sb, l2win
                nw = fw + sw
                nblk = lm["nblk"]
                gath, blocks = lm["gath"], lm["blocks"]
                er_ks, agg_tiles = lm["er_ks"], lm["agg_tiles"]
                bplanes = lm["bplanes"]

                gblocks = {}
                for (b, w, g0, nt) in gath:
                    gblocks.setdefault(b, []).append((w, g0, nt))

                with ExitStack() as ectx:
                    pool = ectx.enter_context(
                        tc.tile_pool(name=f"edge{layer}", bufs=2))
                    pps = ectx.enter_context(
                        tc.tile_pool(name=f"eps{layer}", bufs=2,
                                     space="PSUM"))
                    ppost = ectx.enter_context(
                        tc.tile_pool(name=f"post{layer}", bufs=2))
                    pps2 = ectx.enter_context(
                        tc.tile_pool(name=f"ep2{layer}", bufs=2,
                                     space="PSUM"))
                    ppsE = ectx.enter_context(
                        tc.tile_pool(name=f"epE{layer}", bufs=2,
                                     space="PSUM"))
                    for b in range(nblk):
                        t0, tb = blocks[b]
                        assert tb * sw <= 512, (tb, sw)
                        segs = gblocks[b]
                        gt = pool.tile([128, tb, rw], gdt, tag="gt")
                        ii = pool.tile([128, tb * 8], i16, tag="gi")
                        nc.sync.dma_start(
                            ii[:], gidx_d[:, t0 * 8:(t0 + tb) * 8])
                        if bulk:
                            tabsrc = htabB if layer == 1 else l2tabB
                            nc.sync.dma_start(
                                gt[:],
                                tabsrc[0:128 * tb, :].rearrange(
                                    "(p s) r -> p s r", p=128))
                        for (w, g0, nt) in segs:
                            if bulk:
                                continue
                            t_, lo, hi = twin[w]
                            tab = t_[lo:hi, :]
                            for s0 in range(0, nt, GMAX):
                                sn = min(GMAX, nt - s0)
                                a = g0 - t0 + s0
                                nc.gpsimd.dma_gather(
                                    out_ap=gt[:, a:a + sn, :],
                                    in_ap=tab,
                                    idxs_ap=ii[:, a * 8:(a + sn) * 8],
                                    num_idxs=sn * 128,
                                    num_idxs_reg=sn * 128, elem_size=rw,
                                    queue_num=next_q())
                        # one-hot builds (all-bf16 SBUF -> DVE 4x mode)
                        slF = pool.tile([128, tb * 128], bf16, tag="slF")
                        nc.sync.dma_start(
                            slF[:],
                            slotF_d[0:1, t0 * 128:(t0 + tb) * 128]
                            .broadcast_to([128, tb * 128]))
                        BT = pool.tile([128, BLK, tb * 128], bf16, tag="BT")
                        sl = pool.tile([128, tb], f32, tag="sl")
                        nc.sync.dma_start(sl[:], slotB_d[:, t0:t0 + tb])
                        for kk in range(BLK):
                            rng = lm["plane_rng"][b][kk]
                            if rng is None:
                                continue
                            lo, hi = rng
                            nc.vector.tensor_scalar(
                                out=BT[:, kk, lo * 128:hi * 128],
                                in0=slF[:, lo * 128:hi * 128],
                                scalar1=iota_pk[kk][:], scalar2=None,
                                op0=OP.is_equal)
                        # per-edge er via transposed one-hot matmul
                        psE = ppsE.tile([128, tb * sw], f32, tag="psE")
                        for t in range(tb):
                            ks = er_ks[b][t]
                            for i, kk in enumerate(ks):
                                k = b * BLK + kk
                                nc.tensor.matmul(
                                    psE[:, t * sw:(t + 1) * sw],
                                    lhsT=BT[:, kk, t * 128:(t + 1) * 128],
                                    rhs=er_sb[:, k, :],
                                    start=(i == 0), stop=(i == len(ks) - 1))
                        # one-hot B: materialize slot-replicated tensor once
                        # (2x_2p copy), then per-kk is_equal at 2x_1p
                        slrep = pool.tile([128, tb, 128], bf16, tag="slrep")
                        nc.vector.tensor_scalar_add(
                            slrep[:],
                            sl[:, :].unsqueeze(2)
                            .broadcast_to([128, tb, 128]), 0.0)
                        B = pool.tile([128, tb, BLK * 128], bf16, tag="B")
                        for kk in range(BLK):
                            lo, hi = bplanes[b][kk]
                            nc.vector.tensor_tensor(
                                out=B[:, lo:hi, kk * 128:(kk + 1) * 128],
                                in0=iota2b_sb[:, kk * 128:(kk + 1) * 128]
                                .unsqueeze(1)
                                .broadcast_to([128, hi - lo, 128]),
                                in1=slrep[:, lo:hi, :],
                                op=OP.is_equal)
                        ex = pool.tile([128, tb, sw], f32, tag="ex")
                        if layer == 1:
                            # el recompute: tmp = gt_h * al ((d s) bf16, 2x)
                            # then tree-fold over d down to [tb, sw]
                            tmp = pool.tile([128, tb, hd], bf16, tag="tmpel")
                            nc.vector.tensor_mul(
                                tmp[:], gt[:, :, 0:hd],
                                al_sb[:].unsqueeze(1)
                                .broadcast_to([128, tb, hd]))
                            for half in (hd // 2, hd // 4, hd // 8):
                                nc.vector.tensor_add(
                                    tmp[:, :, 0:half], tmp[:, :, 0:half],
                                    tmp[:, :, half:2 * half])
                            nc.vector.tensor_add(
                                ex[:], tmp[:, :, 0:sw],
                                tmp[:, :, sw:2 * sw])
                            nc.vector.tensor_add(
                                ex[:], ex[:],
                                psE[:].rearrange("p (t s) -> p t s", s=sw))
                        else:
                            nc.vector.tensor_add(
                                ex[:], gt[:, :, fw + 1:fw + 2],
                                psE[:].rearrange("p (t s) -> p t s", s=sw))
                        nc.vector.scalar_tensor_tensor(
                            out=ex[:], in0=ex[:], scalar=NEG_SLOPE,
                            in1=ex[:], op0=OP.mult, op1=OP.max)
                        comb = pool.tile([128, tb, nw], bf16, tag="comb")
                        if layer == 1:
                            nc.scalar.activation(comb[:, :, fw:fw + sw],
                                                 ex[:], AF.Exp)
                            nc.vector.tensor_mul(
                                comb[:, :, 0:fw].rearrange(
                                    "p t (d s) -> p t d s", s=sw),
                                gt[:, :, 0:fw].rearrange(
                                    "p t (d s) -> p t d s", s=sw),
                                comb[:, :, fw:fw + sw].unsqueeze(2)
                                .broadcast_to([128, tb, fw // sw, sw]))
                        else:
                            # comb = ex * [hh | 1]; denominator via ones col
                            nc.scalar.activation(ex[:], ex[:], AF.Exp)
                            nc.vector.tensor_mul(
                                comb[:], gt[:, :, 0:nw],
                                ex[:].broadcast_to([128, tb, nw]))
                        for kk in range(BLK):
                            k = b * BLK + kk
                            if k >= nchunk:
                                break
                            tl = agg_tiles[b][kk]
                            ps = pps.tile([128, nw], f32, tag="agg")
                            for j, t in enumerate(tl):
                                nc.tensor.matmul(
                                    ps[:], lhsT=B[:, t - t0,
                                                  kk * 128:(kk + 1) * 128],
                                    rhs=comb[:, t - t0, :],
                                    start=(j == 0), stop=(j == len(tl) - 1))
                            if layer == 1:
                                post_chunk_l1(k, ps, ppost, pps2)
                            else:
                                post_chunk_l2(k, ps, ppost)
                        if mid_blk is not None and b == mid_blk:
                            mid_cb()

            for _rep in range(krep):
                with ExitStack() as actx:
                    phase_a(actx)
                if kphase != "A":
                    if rowsA:
                        edge_phase(1, mid_blk=hchunk // BLK - 1,
                                   mid_cb=lambda: ag(l2shardA, l2tabA))
                    else:
                        edge_phase(1)
                    ag(l2shardB, l2tabB)
                    if kphase != "L1":
                        edge_phase(2)

    nc_.compile()
    return nc_


# ------------------------------------------------------------------ driver
def make_in_maps(meta, feat, W1, al1, ar1, b1, W2, al2, ar2, b2):
    import ml_dtypes
    bf16 = ml_dtypes.bfloat16
    sh, n_pad = meta["sh"], meta["n_pad"]
    n = meta["n_nodes"]
    feat_pad = np.zeros((n_pad, feat.shape[1]), np.float32)
    feat_pad[:n] = feat
    # fused [W1 | W1@ar]: er is linear in feat.  Hidden columns are stored
    # head-INTERLEAVED ((d s): col = d*heads + s) so per-head broadcasts in
    # the edge phase have a packed last dim (DVE 2x perf mode).
    heads, hid = al1.shape
    # new[:, d*H+s] = old[:, s*hid+d]:  new = old[:, inv] with inv[d*H+s]=s*hid+d
    inv = (np.arange(heads)[None, :] * hid
           + np.arange(hid)[:, None]).reshape(-1)
    w1r = W1.reshape(-1, heads, hid)
    w1ar = np.einsum("fhd,hd->fh", w1r, ar1)
    W1a = np.concatenate([W1[:, inv], w1ar], axis=1).astype(np.float32)
    # al in (d s) order: al_ds[d*H+s] = al1[s, d]
    alrep = np.tile(al1.T.reshape(1, -1), (128, 1)).astype(bf16)
    b1rep = np.tile(b1[inv].reshape(1, -1), (128, 1)).astype(np.float32)
    W2a = np.concatenate(
        [W2, W2 @ al2.reshape(-1, 1), W2 @ ar2.reshape(-1, 1)],
        axis=1).astype(np.float32)[inv, :]
    b2rep = np.tile(b2.reshape(1, -1), (128, 1)).astype(np.float32)
    el = meta["el"]
    in_maps = []
    for c in range(NC):
        in_maps.append({
            "featT": np.ascontiguousarray(
                feat_pad[c * sh:(c + 1) * sh].T).astype(bf16),
            "W1": W1a.astype(bf16),
            "b1rep": b1rep, "alrep": alrep,
            "W2a": W2a, "b2rep": b2rep,
            "gidx": _wrap16(el["gidx"][c]),
            "slotB": np.ascontiguousarray(
                el["slotB"][c].reshape(-1, 128).T).astype(np.float32),
            "slotF": np.maximum(el["slotB"][c], 0.0).reshape(1, -1)
            .astype(bf16),
        })
    return in_maps


class Runner:
    """Builds the SPMD program once; exposes a repeatable timed executor."""

    def __init__(self, meta, f_in):
        self.meta = meta
        self.nc = build_program(meta, f_in, HID, HEADS, N_CLASSES)
        self._fn = None

    def _lower(self):
        import jax
        import numpy as _np
        from jax.sharding import Mesh, PartitionSpec
        from jax.experimental.shard_map import shard_map
        from concourse import mybir
        from concourse.bass2jax import _bass_exec_p, install_neuronx_cc_hook

        install_neuronx_cc_hook()
        nc = self.nc
        in_names, out_names, out_avals, zero_outs = [], [], [], []
        partition_name = (nc.partition_id_tensor.name
                          if nc.partition_id_tensor else None)
        for alloc in nc.m.functions[0].allocations:
            if not isinstance(alloc, mybir.MemoryLocationSet):
                continue
            name = alloc.memorylocations[0].name
            if alloc.kind == "ExternalInput":
                if name != partition_name:
                    in_names.append(name)
            elif alloc.kind == "ExternalOutput":
                shape = tuple(alloc.tensor_shape)
                dtype = mybir.dt.np(alloc.dtype)
                out_names.append(name)
                out_avals.append(jax.core.ShapedArray(shape, dtype))
                zero_outs.append(_np.zeros(shape, dtype))
        n_params = len(in_names)
        n_outs = len(out_avals)
        all_in_names = list(in_names) + list(out_names)
        if partition_name is not None:
            all_in_names.append(partition_name)

        def _body(*args):
            ins = list(args[:n_params])
            zouts = list(args[n_params:])
            operands = ins + zouts
            if partition_name is not None:
                from concourse.bass2jax import partition_id_tensor
                operands.append(partition_id_tensor())
            outs = _bass_exec_p.bind(
                *operands, out_avals=tuple(out_avals),
                in_names=tuple(all_in_names), out_names=tuple(out_names),
                lowering_input_output_aliases=(),
                sim_require_finite=False, sim_require_nnan=False, nc=nc)
            return tuple(outs)

        devices = jax.devices()[:NC]
        mesh = Mesh(_np.asarray(devices), ("core",))
        in_specs = (PartitionSpec("core"),) * (n_params + n_outs)
        out_specs = (PartitionSpec("core"),) * n_outs
        self._fn = jax.jit(
            shard_map(_body, mesh=mesh, in_specs=in_specs,
                      out_specs=out_specs, check_rep=False),
            keep_unused=True)
        self._in_names = in_names
        self._out_names = out_names
        self._out_avals = out_avals
        self._zero_outs = zero_outs
        self._mesh = mesh
        self._in_specs = in_specs

    def prepare(self, in_maps):
        import jax
        import numpy as _np
        from jax.sharding import NamedSharding, PartitionSpec
        if self._fn is None:
            self._lower()
        concat_in = [
            _np.concatenate([in_maps[c][name] for c in range(NC)], axis=0)
            for name in self._in_names]
        concat_zeros = [
            _np.zeros((NC * z.shape[0], *z.shape[1:]), z.dtype)
            for z in self._zero_outs]
        shd = NamedSharding(self._mesh, PartitionSpec("core"))
        self._args = [jax.device_put(a, shd) for a in concat_in + concat_zeros]
        jax.block_until_ready(self._args)

    def run(self):
        import jax
        out = self._fn(*self._args)
        out = jax.block_until_ready(out)
        import numpy as _np
        res = _np.asarray(out[self._out_names.index("out")])
        shp = self._out_avals[self._out_names.index("out")].shape
        res = res.reshape(NC, *shp)
        n, sh = self.meta["n_nodes"], self.meta["sh"]
        parts = [res[c][:min(sh, max(0, n - c * sh))] for c in range(NC)]
        return _np.concatenate(parts, axis=0)


_RUNNER = None


def get_runner(feat, src, dst):
    global _RUNNER
    n, f_in = feat.shape
    meta = host_prep(np.asarray(src, np.int32), np.asarray(dst, np.int32),
                     n_nodes=n)
    _RUNNER = Runner(meta, f_in)
    return _RUNNER


def kernel(feat, src, dst, W1, al1, ar1, b1, W2, al2, ar2, b2):
    feat = np.asarray(feat, dtype=np.float32)
    src = np.asarray(src, dtype=np.int32)
    dst = np.asarray(dst, dtype=np.int32)
    args = [np.asarray(x, np.float32)
            for x in (W1, al1, ar1, b1, W2, al2, ar2, b2)]
    r = _RUNNER if _RUNNER is not None else get_runner(feat, src, dst)
    in_maps = make_in_maps(r.meta, feat, *args)
    r.prepare(in_maps)
    return r.run()


kernel.last_exec_time_ns = None

